# revision 1
# baseline (speedup 1.0000x reference)
"""Trainium2 Bass kernel: LocalCausalTransformerBlock (window-3 causal attention).

Sharding: 8-way sequence-parallel. B=2 x N=2048 = 4096 tokens -> 8 chunks of
512 tokens (4 chunks per batch row). Each core gets its 512 tokens plus a
2-token halo (the preceding tokens of the same sequence) so the window-3
causal attention needs no cross-core communication. Weights are replicated.

Device layout: activations live "transposed" (channels on partitions, tokens
on the free axis) so every matmul contracts over partitions and the +-1/+-2
token shifts of the local attention are plain free-axis offsets.

Host-side folds: LayerNorm gamma/beta are folded into the following matmul
weights/bias; the attention scale (1/sqrt(64)) is folded into the Q columns
of qkv_w/qkv_b. Matmul inputs are cast to bf16 (fp32 accumulate in PSUM);
LayerNorm stats, softmax and both residual streams stay fp32.
"""

import sys

for _p in ("/opt/trn_rl_repo",):
    if _p not in sys.path:
        sys.path.insert(0, _p)

import numpy as np
import ml_dtypes

P = 128
D = 1024
H = 16
HD = 64
H3 = 3 * D
HID = 4096
T = 512            # real tokens per core
TH = T + 2         # with 2-token halo (halo stored first)
NCORE = 8
EPS = 1e-5
NEG = -1e30
BF = ml_dtypes.bfloat16

_CACHE: dict = {}


def _build_program():
    import concourse.bass as bass
    import concourse.tile as tile
    from concourse import bacc, mybir
    from contextlib import ExitStack

    f32 = mybir.dt.float32
    bf16 = mybir.dt.bfloat16
    ALU = mybir.AluOpType
    ACT = mybir.ActivationFunctionType

    nc = bacc.Bacc()

    xh_d = nc.declare_dram_parameter("xh", [2, D], f32, isOutput=False)
    xm_d = nc.declare_dram_parameter("xm", [T, D], f32, isOutput=False)
    qkvw_d = nc.declare_dram_parameter("qkvw", [D, H3], bf16, isOutput=False)
    projw_d = nc.declare_dram_parameter("projw", [D, D], bf16, isOutput=False)
    fc1w_d = nc.declare_dram_parameter("fc1w", [D, HID], bf16, isOutput=False)
    fc2w_d = nc.declare_dram_parameter("fc2w", [HID, D], bf16, isOutput=False)
    qkvb_d = nc.declare_dram_parameter("qkvb", [P, 24], f32, isOutput=False)
    projb_d = nc.declare_dram_parameter("projb", [P, 8], f32, isOutput=False)
    fc1b_d = nc.declare_dram_parameter("fc1b", [P, 32], f32, isOutput=False)
    fc2b_d = nc.declare_dram_parameter("fc2b", [P, 8], f32, isOutput=False)
    idf_d = nc.declare_dram_parameter("idf", [P, P], f32, isOutput=False)
    idb_d = nc.declare_dram_parameter("idb", [P, P], bf16, isOutput=False)
    hmask_d = nc.declare_dram_parameter("hmask", [P, 8 * H], bf16, isOutput=False)
    emask_d = nc.declare_dram_parameter("emask", [H, 8 * P], bf16, isOutput=False)
    smask_d = nc.declare_dram_parameter("smask", [H, 3 * T], f32, isOutput=False)
    out_d = nc.declare_dram_parameter("out", [T, D], f32, isOutput=True)

    with tile.TileContext(nc) as tc, ExitStack() as ctx:
        # ---- program-lifetime pools ----
        const = ctx.enter_context(tc.tile_pool(name="const", bufs=1))
        acts = ctx.enter_context(tc.tile_pool(name="acts", bufs=1))
        ln_pool = ctx.enter_context(tc.tile_pool(name="ln", bufs=3))
        tp_ps = ctx.enter_context(tc.tile_pool(name="tp_ps", bufs=3, space="PSUM"))
        mm_ps = ctx.enter_context(tc.tile_pool(name="mm_ps", bufs=5, space="PSUM"))

        idf = const.tile([P, P], f32, tag="c", name="idf")
        nc.sync.dma_start(idf[:], idf_d[:])
        idb = const.tile([P, P], bf16, tag="c2", name="idb")
        nc.sync.dma_start(idb[:], idb_d[:])
        hmask = const.tile([P, 8 * H], bf16, tag="c3", name="hmask")
        nc.sync.dma_start(hmask[:], hmask_d[:])
        emask = const.tile([H, 8 * P], bf16, tag="c4", name="emask")
        nc.sync.dma_start(emask[:], emask_d[:])
        smask = const.tile([H, 3 * T], f32, tag="c5", name="smask")
        nc.sync.dma_start(smask[:], smask_d[:])
        qkvb = const.tile([P, 24], f32, tag="c6", name="qkvb")
        nc.sync.dma_start(qkvb[:], qkvb_d[:])
        projb = const.tile([P, 8], f32, tag="c7", name="projb")
        nc.sync.dma_start(projb[:], projb_d[:])
        fc1b = const.tile([P, 32], f32, tag="c8", name="fc1b")
        nc.sync.dma_start(fc1b[:], fc1b_d[:])
        fc2b = const.tile([P, 8], f32, tag="c9", name="fc2b")
        nc.sync.dma_start(fc2b[:], fc2b_d[:])

        # activations alive into the MLP phases
        x2t = acts.tile([P, 4 * D], f32, tag="x2t", name="x2t")
        x2lnT = acts.tile([P, 8 * T], bf16, tag="x2lnT", name="x2lnT")
        hT = acts.tile([P, 32 * T], bf16, tag="hT", name="hT")

        def layernorm_T(src_ap, s, dstT, dst_col, dst_stride):
            stat = ln_pool.tile([s, 12], f32, tag=f"lnstat{s}", name=f"st{s}")
            nc.vector.bn_stats(stat[:, 0:6], src_ap[:, 0:512])
            nc.vector.bn_stats(stat[:, 6:12], src_ap[:, 512:1024])
            mv = ln_pool.tile([s, 2], f32, tag=f"lnmv{s}", name=f"mv{s}")
            nc.vector.bn_aggr(mv[:], stat[:])
            vpe = ln_pool.tile([s, 1], f32, tag=f"lnvpe{s}", name=f"vpe{s}")
            nc.vector.tensor_scalar_add(vpe[:], mv[:, 1:2], EPS)
            std = ln_pool.tile([s, 1], f32, tag=f"lnstd{s}", name=f"sd{s}")
            nc.scalar.activation(std[:], vpe[:], ACT.Sqrt)
            rstd = ln_pool.tile([s, 1], f32, tag=f"lnrstd{s}", name=f"rs{s}")
            nc.vector.reciprocal(rstd[:], std[:])
            nmr = ln_pool.tile([s, 1], f32, tag=f"lnnmr{s}", name=f"nm{s}")
            nc.vector.scalar_tensor_tensor(
                nmr[:], mv[:, 0:1], -1.0, rstd[:], ALU.mult, ALU.mult
            )
            xln = ln_pool.tile([s, D], bf16, tag=f"lnout{s}", name=f"xo{s}")
            nc.scalar.activation(
                xln[:], src_ap[:], ACT.Identity, bias=nmr[:, 0:1], scale=rstd[:, 0:1]
            )
            for ch in range(8):
                tp = tp_ps.tile([P, s], bf16, tag="tp", name=f"tpl{s}_{ch}")
                nc.tensor.transpose(tp[:], xln[:, ch * P:(ch + 1) * P], idb[0:s, 0:s])
                c0 = ch * dst_stride + dst_col
                nc.vector.tensor_copy(dstT[:, c0:c0 + s], tp[:])

        with tc.tile_pool(name="p1", bufs=1) as p1:
            xt = p1.tile([P, 4 * D], f32, tag="xt", name="xt")
            xh = p1.tile([2, D], f32, tag="xh", name="xh")
            xlnT = p1.tile([P, 8 * TH], bf16, tag="xlnT", name="xlnT")
            qT = p1.tile([P, 8 * T], bf16, tag="qT", name="qT")
            kT = p1.tile([P, 8 * TH], bf16, tag="kT", name="kT")
            vT = p1.tile([P, 8 * TH], f32, tag="vT", name="vT")

            for ti in range(4):
                nc.sync.dma_start(xt[:, ti * D:(ti + 1) * D],
                                  xm_d[ti * P:(ti + 1) * P, :])
            nc.sync.dma_start(xh[:], xh_d[:])

            # ---- LN1 (halo + 4 token tiles) ----
            layernorm_T(xh[:], 2, xlnT, 0, TH)
            for ti in range(4):
                layernorm_T(xt[:, ti * D:(ti + 1) * D], P, xlnT, 2 + ti * P, TH)

            # ---- QKV ----
            with tc.tile_pool(name="wq", bufs=1) as wq_pool:
                qslab = []
                for c in range(8):
                    s = wq_pool.tile([P, H3], bf16, tag=f"qw{c}", name=f"qw{c}")
                    nc.sync.dma_start(s[:], qkvw_d[c * P:(c + 1) * P, :])
                    qslab.append(s)

                for j in range(24):
                    ps = mm_ps.tile([P, T], f32, tag="mm", name=f"qkv{j}")
                    for c in range(8):
                        nc.tensor.matmul(
                            ps[:], qslab[c][:, j * P:(j + 1) * P],
                            xlnT[:, c * TH + 2:c * TH + TH],
                            start=(c == 0), stop=(c == 7),
                        )
                    bias = qkvb[:, j:j + 1]
                    if j < 8:
                        dst = qT[:, j * T:(j + 1) * T]
                    elif j < 16:
                        dst = kT[:, (j - 8) * TH + 2:(j - 8) * TH + TH]
                    else:
                        dst = vT[:, (j - 16) * TH + 2:(j - 16) * TH + TH]
                    nc.scalar.activation(dst, ps[:], ACT.Identity, bias=bias)
                    if j >= 8:  # halo K/V columns
                        ph = tp_ps.tile([P, 2], f32, tag="tp", name=f"halo{j}")
                        for c in range(8):
                            nc.tensor.matmul(
                                ph[:], qslab[c][:, j * P:(j + 1) * P],
                                xlnT[:, c * TH:c * TH + 2],
                                start=(c == 0), stop=(c == 7),
                            )
                        if j < 16:
                            hdst = kT[:, (j - 8) * TH:(j - 8) * TH + 2]
                        else:
                            hdst = vT[:, (j - 16) * TH:(j - 16) * TH + 2]
                        nc.scalar.activation(hdst, ph[:], ACT.Identity, bias=bias)

            # ---- attention ----
            with tc.tile_pool(name="p3", bufs=1) as p3:
                attnT = p3.tile([P, 8 * T], bf16, tag="attnT", name="attnT")
                with tc.tile_pool(name="p3b", bufs=1) as p3b:
                    s_sb = p3b.tile([H, 3 * T], f32, tag="s_sb", name="s_sb")
                    for w in range(3):
                        sc = mm_ps.tile([H, T], f32, tag="mm", name=f"sc{w}")
                        for ch in range(8):
                            e = p3b.tile([P, T], bf16, tag="e", bufs=3, name=f"e{w}_{ch}")
                            nc.vector.tensor_mul(
                                e[:], qT[:, ch * T:(ch + 1) * T],
                                kT[:, ch * TH + 2 - w:ch * TH + TH - w],
                            )
                            nc.tensor.matmul(
                                sc[:], hmask[:, ch * H:(ch + 1) * H], e[:],
                                start=(ch == 0), stop=(ch == 7),
                            )
                        nc.vector.tensor_add(
                            s_sb[:, w * T:(w + 1) * T], sc[:],
                            smask[:, w * T:(w + 1) * T]
                        )
                    mx = p3b.tile([H, T], f32, tag="mx", name="mx")
                    mx2 = p3b.tile([H, T], f32, tag="mx2", name="mx2")
                    nc.vector.tensor_max(mx[:], s_sb[:, 0:T], s_sb[:, T:2 * T])
                    nc.vector.tensor_max(mx2[:], mx[:], s_sb[:, 2 * T:3 * T])
                    st2 = p3b.tile([H, 3 * T], f32, tag="st2", name="st2")
                    et = p3b.tile([H, 3 * T], f32, tag="et", name="et")
                    for w in range(3):
                        nc.vector.tensor_sub(st2[:, w * T:(w + 1) * T],
                                             s_sb[:, w * T:(w + 1) * T], mx2[:])
                        nc.scalar.activation(et[:, w * T:(w + 1) * T],
                                             st2[:, w * T:(w + 1) * T], ACT.Exp)
                    z0 = p3b.tile([H, T], f32, tag="z0", name="z0")
                    z1 = p3b.tile([H, T], f32, tag="z1", name="z1")
                    rz = p3b.tile([H, T], f32, tag="rz", name="rz")
                    nc.vector.tensor_add(z0[:], et[:, 0:T], et[:, T:2 * T])
                    nc.vector.tensor_add(z1[:], z0[:], et[:, 2 * T:3 * T])
                    nc.vector.reciprocal(rz[:], z1[:])
                    pw = p3b.tile([H, 3 * T], bf16, tag="pw", name="pw")
                    for w in range(3):
                        nc.vector.tensor_mul(pw[:, w * T:(w + 1) * T],
                                             et[:, w * T:(w + 1) * T], rz[:])

                    for ch in range(8):
                        avs = []
                        for w in range(3):
                            bc = mm_ps.tile([P, T], f32, tag="mm", name=f"bc{ch}_{w}")
                            nc.tensor.matmul(
                                bc[:], emask[:, ch * P:(ch + 1) * P],
                                pw[:, w * T:(w + 1) * T],
                                start=True, stop=True,
                            )
                            av = p3b.tile([P, T], f32, tag="av", bufs=4,
                                          name=f"av{ch}_{w}")
                            nc.vector.tensor_mul(
                                av[:], bc[:], vT[:, ch * TH + 2 - w:ch * TH + TH - w]
                            )
                            avs.append(av)
                        av01 = p3b.tile([P, T], f32, tag="av01", bufs=2,
                                        name=f"av01_{ch}")
                        nc.vector.tensor_add(av01[:], avs[0][:], avs[1][:])
                        nc.vector.tensor_add(attnT[:, ch * T:(ch + 1) * T],
                                             av01[:], avs[2][:])

                # ---- proj + residual 1 + LN2 ----
                with tc.tile_pool(name="p5", bufs=1) as p5:
                    pslab = []
                    for c in range(8):
                        s = p5.tile([P, D], bf16, tag=f"pw{c}", name=f"pjw{c}")
                        nc.sync.dma_start(s[:], projw_d[c * P:(c + 1) * P, :])
                        pslab.append(s)
                    yT = p5.tile([P, 8 * T], f32, tag="yT", name="yT")
                    for j in range(8):
                        ps = mm_ps.tile([P, T], f32, tag="mm", name=f"pj{j}")
                        for c in range(8):
                            nc.tensor.matmul(
                                ps[:], pslab[c][:, j * P:(j + 1) * P],
                                attnT[:, c * T:(c + 1) * T],
                                start=(c == 0), stop=(c == 7),
                            )
                        nc.scalar.activation(yT[:, j * T:(j + 1) * T], ps[:],
                                             ACT.Identity, bias=projb[:, j:j + 1])
                    for ti in range(4):
                        for ch in range(8):
                            tp = tp_ps.tile([P, P], f32, tag="tp", name=f"tpy{ti}_{ch}")
                            nc.tensor.transpose(
                                tp[:], yT[:, ch * T + ti * P:ch * T + (ti + 1) * P],
                                idf[:])
                            nc.vector.tensor_add(
                                x2t[:, ti * D + ch * P:ti * D + (ch + 1) * P],
                                xt[:, ti * D + ch * P:ti * D + (ch + 1) * P], tp[:],
                            )
                        layernorm_T(x2t[:, ti * D:(ti + 1) * D], P, x2lnT, ti * P, T)

        # ---- MLP fc1 + gelu ----
        with tc.tile_pool(name="w1", bufs=1) as w1_pool:
            f1slab = []
            for c in range(8):
                s = w1_pool.tile([P, HID], bf16, tag=f"f1w{c}", name=f"f1w{c}")
                nc.sync.dma_start(s[:], fc1w_d[c * P:(c + 1) * P, :])
                f1slab.append(s)
            for j in range(32):
                ps = mm_ps.tile([P, T], f32, tag="mm", name=f"f1{j}")
                for c in range(8):
                    nc.tensor.matmul(
                        ps[:], f1slab[c][:, j * P:(j + 1) * P],
                        x2lnT[:, c * T:(c + 1) * T],
                        start=(c == 0), stop=(c == 7),
                    )
                nc.scalar.activation(hT[:, j * T:(j + 1) * T], ps[:], ACT.Gelu,
                                     bias=fc1b[:, j:j + 1])

        # ---- fc2 + residual 2 + store ----
        with tc.tile_pool(name="w2", bufs=1) as w2_pool:
            outt = w2_pool.tile([P, 4 * D], f32, tag="outt", name="outt")
            mlp_written = set()
            for jg in range(2):
                pss = [mm_ps.tile([P, T], f32, tag="mm", name=f"mm4_{jg}_{j}")
                       for j in range(4)]
                for c in range(32):
                    slab = w2_pool.tile([P, D], bf16, tag="f2w", bufs=6,
                                        name=f"f2w{jg}_{c}")
                    nc.sync.dma_start(slab[:], fc2w_d[c * P:(c + 1) * P, :])
                    for j in range(4):
                        nc.tensor.matmul(
                            pss[j][:], slab[:, (jg * 4 + j) * P:(jg * 4 + j + 1) * P],
                            hT[:, c * T:(c + 1) * T],
                            start=(c == 0), stop=(c == 31),
                        )
                for j in range(4):
                    jj = jg * 4 + j
                    mlpt = w2_pool.tile([P, T], f32, tag="mlpt", bufs=2,
                                        name=f"mlpt{jj}")
                    nc.scalar.activation(mlpt[:], pss[j][:], ACT.Identity,
                                         bias=fc2b[:, jj:jj + 1])
                    for ti in range(4):
                        tp = tp_ps.tile([P, P], f32, tag="tp", name=f"tpm{jj}_{ti}")
                        nc.tensor.transpose(tp[:], mlpt[:, ti * P:(ti + 1) * P],
                                            idf[:])
                        nc.vector.tensor_add(
                            outt[:, ti * D + jj * P:ti * D + (jj + 1) * P],
                            x2t[:, ti * D + jj * P:ti * D + (jj + 1) * P], tp[:],
                        )
            for ti in range(4):
                nc.sync.dma_start(out_d[ti * P:(ti + 1) * P, :],
                                  outt[:, ti * D:(ti + 1) * D])

    if not nc.is_finalized():
        nc.finalize()
    return nc


def _host_inputs(x, qkv_w, qkv_b, proj_w, proj_b, g1, b1, g2, b2,
                 fc1_w, fc1_b, fc2_w, fc2_b):
    """Build the 8 per-core input maps (fold LN affine + attn scale)."""
    scale = HD ** -0.5
    qkvw_eff = (qkv_w * g1[:, None]).astype(np.float32).copy()
    qkvb_eff = (qkv_b + b1 @ qkv_w).astype(np.float32).copy()
    qkvw_eff[:, 0:D] *= scale
    qkvb_eff[0:D] *= scale
    fc1w_eff = (fc1_w * g2[:, None]).astype(np.float32)
    fc1b_eff = (fc1_b + b2 @ fc1_w).astype(np.float32)

    common = {
        "qkvw": qkvw_eff.astype(BF),
        "projw": proj_w.astype(BF),
        "fc1w": fc1w_eff.astype(BF),
        "fc2w": fc2_w.astype(BF),
        "qkvb": qkvb_eff.reshape(24, P).T.copy(),
        "projb": proj_b.astype(np.float32).reshape(8, P).T.copy(),
        "fc1b": fc1b_eff.reshape(32, P).T.copy(),
        "fc2b": fc2_b.astype(np.float32).reshape(8, P).T.copy(),
        "idf": np.eye(P, dtype=np.float32),
        "idb": np.eye(P, dtype=np.float32).astype(BF),
    }
    hm = np.zeros((P, 8, H), np.float32)
    for c in range(P):
        for ch in range(8):
            hm[c, ch, 2 * ch + c // HD] = 1.0
    common["hmask"] = hm.reshape(P, 8 * H).astype(BF)
    em = np.zeros((H, 8, P), np.float32)
    for ch in range(8):
        for m in range(P):
            em[2 * ch + m // HD, ch, m] = 1.0
    common["emask"] = em.reshape(H, 8 * P).astype(BF)

    sm0 = np.zeros((H, 3, T), np.float32)
    smq0 = sm0.copy()
    smq0[:, 1, 0] = NEG
    smq0[:, 2, 0:2] = NEG

    in_maps = []
    for core in range(NCORE):
        b, q = divmod(core, 4)
        xm = np.ascontiguousarray(x[b, q * T:(q + 1) * T, :], dtype=np.float32)
        if q == 0:
            xhv = np.zeros((2, D), np.float32)
        else:
            xhv = np.ascontiguousarray(x[b, q * T - 2:q * T, :], dtype=np.float32)
        m = dict(common)
        m["xm"] = xm
        m["xh"] = xhv
        m["smask"] = (smq0 if q == 0 else sm0).reshape(H, 3 * T).copy()
        in_maps.append(m)
    return in_maps


def kernel(**inputs) -> np.ndarray:
    from concourse.bass_utils import run_bass_kernel_spmd

    if "nc" not in _CACHE:
        _CACHE["nc"] = _build_program()
    nc = _CACHE["nc"]
    in_maps = _host_inputs(**inputs)
    res = run_bass_kernel_spmd(nc, in_maps, list(range(NCORE)))
    outs = res.results
    full = np.zeros((2, 2048, D), np.float32)
    for core in range(NCORE):
        b, q = divmod(core, 4)
        full[b, q * T:(q + 1) * T, :] = outs[core]["out"]
    return full



# revision 10
# speedup vs baseline: 1.5786x; 1.5786x over previous
"""Trainium2 Bass kernel: LocalCausalTransformerBlock (window-3 causal attention).

Sharding: 8-way sequence-parallel. B=2 x N=2048 = 4096 tokens -> 8 chunks of
512 tokens (4 chunks per batch row). Each core gets its 512 tokens plus a
2-token halo (the preceding tokens of the same sequence) so the window-3
causal attention needs no cross-core communication. Weights are replicated.

v2: the four big matmuls (qkv/proj/fc1/fc2) run in fp8e4m3 with DoubleRow
perf mode (contract 2x128 channels per instruction at 0.5 cycles/row).
Weights are pre-scaled per output column to a power of two near absmax~2 so
e4m3's subnormal range is never hit; the descale rides the eviction's free
multiplicative scalar slot. Where more precision is needed the quantization
residual ("lo") is appended as extra fp8 k-chunks in the same accumulation
group. Attention internals (q/k/v, softmax, AV) stay bf16; LayerNorm stats,
softmax normalizer and both residual streams stay fp32.

Device layout: activations live "transposed" (channels on partitions, tokens
on the free axis) so matmuls contract over partitions and the +-1/+-2 token
shifts of the local attention are free-axis offsets. LayerNorm runs
token-major; PE transposes bridge the layouts, batched 8-to-a-psum-bank with
one wide strided eviction. Softmax needs no max-subtraction (window-3 scores
are small): exp runs directly on the score PSUM and a per-core multiplicative
edge mask zeroes out-of-window columns after exp. 1/sqrt(var+eps) is
exp(-0.5*ln(var+eps)) so the Act table only holds {ln,exp,identity} ->
{gelu,identity}: two table loads. PSUM evictions are spread across
DVE/Pool(gpsimd)/Act to keep PE the only saturated engine.
"""

import sys

for _p in ("/opt/trn_rl_repo",):
    if _p not in sys.path:
        sys.path.insert(0, _p)

import numpy as np
import ml_dtypes

P = 128
D = 1024
H = 16
HD = 64
H3 = 3 * D
HID = 4096
T = 512            # real tokens per core
TH = T + 2         # k/v token axis with 2-token halo (halo stored first)
NCORE = 8
EPS = 1e-5
BF = ml_dtypes.bfloat16
F8 = ml_dtypes.float8_e4m3

# which weights carry the fp8 quantization residual (2x k-chunks)
COMP = {"qkv": True, "proj": True, "fc1": False, "fc2": False}

_CACHE: dict = {}


def _build_program():
    import concourse.bass as bass
    import concourse.tile as tile
    from concourse import bacc, mybir
    from contextlib import ExitStack

    f32 = mybir.dt.float32
    bf16 = mybir.dt.bfloat16
    fp8 = mybir.dt.float8e4
    ALU = mybir.AluOpType
    ACT = mybir.ActivationFunctionType
    DR = mybir.MatmulPerfMode.DoubleRow

    KQ = 16 if COMP["qkv"] else 8
    KP = 16 if COMP["proj"] else 8
    K1 = 16 if COMP["fc1"] else 8
    K2 = 64 if COMP["fc2"] else 32

    nc = bacc.Bacc()

    xh_d = nc.declare_dram_parameter("xh", [2, D], f32, isOutput=False)
    xm_d = nc.declare_dram_parameter("xm", [T, D], f32, isOutput=False)
    qkvw_d = nc.declare_dram_parameter("qkvw", [P, KQ * H3], fp8, isOutput=False)
    projw_d = nc.declare_dram_parameter("projw", [P, KP * D], fp8, isOutput=False)
    fc1w_d = nc.declare_dram_parameter("fc1w", [P, K1 * HID], fp8, isOutput=False)
    fc2w_d = nc.declare_dram_parameter("fc2w", [P, K2 * D], fp8, isOutput=False)
    qkvb_d = nc.declare_dram_parameter("qkvb", [P, 24], f32, isOutput=False)
    qkvs_d = nc.declare_dram_parameter("qkvs", [P, 24], f32, isOutput=False)
    projb_d = nc.declare_dram_parameter("projb", [P, 8], f32, isOutput=False)
    projs_d = nc.declare_dram_parameter("projs", [P, 8], f32, isOutput=False)
    fc1b_d = nc.declare_dram_parameter("fc1b", [P, 32], f32, isOutput=False)
    fc1s_d = nc.declare_dram_parameter("fc1s", [P, 32], f32, isOutput=False)
    fc2b_d = nc.declare_dram_parameter("fc2b", [P, 8], f32, isOutput=False)
    fc2s_d = nc.declare_dram_parameter("fc2s", [P, 8], f32, isOutput=False)
    khs_d = nc.declare_dram_parameter("khs", [P, 32], f32, isOutput=False)
    khb_d = nc.declare_dram_parameter("khb", [P, 32], f32, isOutput=False)
    idb_d = nc.declare_dram_parameter("idb", [P, P], bf16, isOutput=False)
    hmask_d = nc.declare_dram_parameter("hmask", [P, 8 * H], bf16, isOutput=False)
    emask_d = nc.declare_dram_parameter("emask", [H, 8 * P], bf16, isOutput=False)
    emk_d = nc.declare_dram_parameter("emk", [H, 3], bf16, isOutput=False)
    out_d = nc.declare_dram_parameter("out", [T, D], f32, isOutput=True)

    with tile.TileContext(nc) as tc, ExitStack() as ctx:
        # ---- program-lifetime pools ----
        # PSUM budget (8 banks): mm x4, sc x2 (scores <-> fc2), tp x2
        const = ctx.enter_context(tc.tile_pool(name="const", bufs=1))
        acts = ctx.enter_context(tc.tile_pool(name="acts", bufs=1))
        ln_pool = ctx.enter_context(tc.tile_pool(name="ln", bufs=3))
        tp_ps = ctx.enter_context(tc.tile_pool(name="tp_ps", bufs=2, space="PSUM"))
        mm_ps = ctx.enter_context(tc.tile_pool(name="mm_ps", bufs=4, space="PSUM"))
        sc_ps = ctx.enter_context(tc.tile_pool(name="sc_ps", bufs=2, space="PSUM"))

        idb = const.tile([P, P], bf16, tag="c2", name="idb")
        nc.sync.dma_start(idb[:], idb_d[:])
        hmask = const.tile([P, 8 * H], bf16, tag="c3", name="hmask")
        nc.sync.dma_start(hmask[:], hmask_d[:])
        emask = const.tile([H, 8 * P], bf16, tag="c4", name="emask")
        nc.sync.dma_start(emask[:], emask_d[:])
        emk = const.tile([H, 3], bf16, tag="c5", name="emk")
        nc.sync.dma_start(emk[:], emk_d[:])
        qkvb = const.tile([P, 24], f32, tag="c6", name="qkvb")
        nc.sync.dma_start(qkvb[:], qkvb_d[:])
        qkvs = const.tile([P, 24], f32, tag="c6s", name="qkvs")
        nc.sync.dma_start(qkvs[:], qkvs_d[:])
        projb = const.tile([P, 8], f32, tag="c7", name="projb")
        nc.sync.dma_start(projb[:], projb_d[:])
        projs = const.tile([P, 8], f32, tag="c7s", name="projs")
        nc.sync.dma_start(projs[:], projs_d[:])
        fc1b = const.tile([P, 32], f32, tag="c8", name="fc1b")
        nc.sync.dma_start(fc1b[:], fc1b_d[:])
        fc1s = const.tile([P, 32], f32, tag="c8s", name="fc1s")
        nc.sync.dma_start(fc1s[:], fc1s_d[:])
        fc2b = const.tile([P, 8], f32, tag="c9", name="fc2b")
        nc.sync.dma_start(fc2b[:], fc2b_d[:])
        fc2s = const.tile([P, 8], f32, tag="c9s", name="fc2s")
        nc.sync.dma_start(fc2s[:], fc2s_d[:])
        khs = const.tile([P, 32], f32, tag="ca", name="khs")
        nc.sync.dma_start(khs[:], khs_d[:])
        khb = const.tile([P, 32], f32, tag="cb", name="khb")
        nc.sync.dma_start(khb[:], khb_d[:])

        # activations alive into the MLP phases
        x2t = acts.tile([P, 4 * D], f32, tag="x2t", name="x2t")
        x2lnT = acts.tile([P, 8, T], fp8, tag="x2lnT", name="x2lnT")
        hT = acts.tile([P, 32, T], fp8, tag="hT", name="hT")

        _ln_site = [0]

        def layernorm_tok(src_ap, s, dstT, dst_off):
            """Token-major LN over s tokens -> fp8, transposed (8 chunks into
            one psum bank, one wide strided eviction) into
            dstT[:, ch, dst_off:dst_off+s]."""
            stat = ln_pool.tile([s, 12], f32, tag=f"lnstat{s}", name=f"st{s}")
            nc.vector.bn_stats(stat[:, 0:6], src_ap[:, 0:512])
            nc.vector.bn_stats(stat[:, 6:12], src_ap[:, 512:1024])
            mv = ln_pool.tile([s, 2], f32, tag=f"lnmv{s}", name=f"mv{s}")
            nc.vector.bn_aggr(mv[:], stat[:])
            vpe = ln_pool.tile([s, 1], f32, tag=f"lnvpe{s}", name=f"vpe{s}")
            nc.vector.tensor_scalar_add(vpe[:], mv[:, 1:2], EPS)
            lnv = ln_pool.tile([s, 1], f32, tag=f"lnlnv{s}", name=f"lv{s}")
            nc.scalar.activation(lnv[:], vpe[:], ACT.Ln)
            rstd = ln_pool.tile([s, 1], f32, tag=f"lnrstd{s}", name=f"rs{s}")
            nc.scalar.activation(rstd[:], lnv[:], ACT.Exp, scale=-0.5)
            nmr = ln_pool.tile([s, 1], f32, tag=f"lnnmr{s}", name=f"nm{s}")
            nc.vector.scalar_tensor_tensor(
                nmr[:], mv[:, 0:1], -1.0, rstd[:], ALU.mult, ALU.mult
            )
            xln = ln_pool.tile([s, D], bf16, tag=f"lnout{s}", name=f"xo{s}")
            nc.gpsimd.tensor_scalar(xln[:], src_ap[:], rstd[:, 0:1], nmr[:, 0:1],
                                    ALU.mult, ALU.add)
            # transpose in bf16 (fp8 transpose is rejected by the backend);
            # the eviction copy casts to fp8
            tpw = tp_ps.tile([P, 8, s], bf16, tag="tp", name=f"tpln{s}")
            for ch in range(8):
                nc.tensor.transpose(tpw[:, ch, :], xln[:, ch * P:(ch + 1) * P],
                                    idb[0:s, 0:s])
            _ln_site[0] += 1
            nc.vector.tensor_copy(dstT[:, :, dst_off:dst_off + s], tpw[:])

        def dr_matmul(ps, wtile, wcol, ncols, moving, kchunks, nmov):
            npairs = kchunks // 2
            for i in range(npairs):
                xc = (2 * i) % nmov
                nc.tensor.matmul(
                    ps,
                    wtile[:, 2 * i:2 * i + 2, wcol:wcol + ncols],
                    moving[:, xc:xc + 2, :],
                    start=(i == 0), stop=(i == npairs - 1),
                    perf_mode=DR,
                )

        with tc.tile_pool(name="p1", bufs=1) as p1:
            xt = p1.tile([P, 4 * D], f32, tag="xt", name="xt")
            xh = p1.tile([2, D], f32, tag="xh", name="xh")
            xlnT = p1.tile([P, 8, T], fp8, tag="xlnT", name="xlnT")
            xlnTh = p1.tile([P, 8, 2], fp8, tag="xlnTh", name="xlnTh")
            qT = p1.tile([P, 8 * T], bf16, tag="qT", name="qT")
            kvT = p1.tile([P, 16 * TH], bf16, tag="kvT", name="kvT")

            for ti in range(4):
                nc.sync.dma_start(xt[:, ti * D:(ti + 1) * D],
                                  xm_d[ti * P:(ti + 1) * P, :])
            nc.sync.dma_start(xh[:], xh_d[:])

            # ---- LN1 (halo + 4 token tiles) ----
            layernorm_tok(xh[:], 2, xlnTh, 0)
            for ti in range(4):
                layernorm_tok(xt[:, ti * D:(ti + 1) * D], P, xlnT, ti * P)

            # ---- QKV ----
            with tc.tile_pool(name="wq", bufs=1) as wq_pool:
                qkvw = wq_pool.tile([P, KQ, H3], fp8, tag="qkvw", name="qkvw")
                nc.sync.dma_start(qkvw[:], qkvw_d[:])

                # halo k/v columns: one psum tile [P, 32] = 16 blocks x 2 cols
                ph = tp_ps.tile([P, 8, 4], f32, tag="tp", name="ph")
                for j in range(16):
                    npairs = KQ // 2
                    for i in range(npairs):
                        xc = (2 * i) % 8
                        nc.tensor.matmul(
                            ph[:, j // 2, (j % 2) * 2:(j % 2) * 2 + 2],
                            qkvw[:, 2 * i:2 * i + 2, D + j * P:D + j * P + P],
                            xlnTh[:, xc:xc + 2, :],
                            start=(i == 0), stop=(i == npairs - 1),
                            perf_mode=DR,
                        )
                # descale (per-feature values vary per block: elementwise
                # tensor), then bias + scatter into the kvT halo columns
                pht = ln_pool.tile([P, 32], f32, tag="pht", name="pht")
                nc.vector.tensor_mul(pht[:], ph[:, :, :], khs[:])
                for j in range(16):
                    nc.gpsimd.tensor_add(
                        kvT[:, j * TH:j * TH + 2], pht[:, 2 * j:2 * j + 2],
                        khb[:, 2 * j:2 * j + 2])

                for j in range(24):
                    ps = mm_ps.tile([P, T], f32, tag="mm", name=f"qkv{j}")
                    dr_matmul(ps[:], qkvw, j * P, P, xlnT, KQ, 8)
                    if j < 8:
                        dst = qT[:, j * T:(j + 1) * T]
                    else:
                        dst = kvT[:, (j - 8) * TH + 2:(j - 8) * TH + TH]
                    nc.scalar.activation(dst, ps[:], ACT.Identity,
                                         bias=qkvb[:, j:j + 1],
                                         scale=qkvs[:, j:j + 1])

            # ---- attention ----
            with tc.tile_pool(name="p3", bufs=1) as p3:
                attnT = p3.tile([P, 8, T], fp8, tag="attnT", name="attnT")
                with tc.tile_pool(name="p3b", bufs=1) as p3b:
                    et = p3b.tile([H, 3 * T], bf16, tag="et", name="et")
                    for w in range(3):
                        sc = sc_ps.tile([H, T], f32, tag="sc", name=f"sc{w}")
                        for ch in range(8):
                            e = p3b.tile([P, T], bf16, tag="e", bufs=3,
                                         name=f"e{w}_{ch}")
                            eng = nc.vector if ch % 2 == 0 else nc.gpsimd
                            eng.tensor_mul(
                                e[:], qT[:, ch * T:(ch + 1) * T],
                                kvT[:, ch * TH + 2 - w:ch * TH + TH - w],
                            )
                            nc.tensor.matmul(
                                sc[:], hmask[:, ch * H:(ch + 1) * H], e[:],
                                start=(ch == 0), stop=(ch == 7),
                            )
                        # exp straight off the psum (no max subtraction)
                        nc.scalar.activation(et[:, w * T:(w + 1) * T], sc[:],
                                             ACT.Exp)
                    # zero out-of-window exp values on each sequence's first chunk
                    nc.gpsimd.tensor_mul(et[:, T:T + 1], et[:, T:T + 1],
                                          emk[:, 0:1])
                    nc.gpsimd.tensor_mul(et[:, 2 * T:2 * T + 2],
                                         et[:, 2 * T:2 * T + 2], emk[:, 1:3])
                    z0 = p3b.tile([H, T], bf16, tag="z0", name="z0")
                    z1 = p3b.tile([H, T], bf16, tag="z1", name="z1")
                    rz = p3b.tile([H, T], bf16, tag="rz", name="rz")
                    nc.gpsimd.tensor_add(z0[:], et[:, 0:T], et[:, T:2 * T])
                    nc.gpsimd.tensor_add(z1[:], z0[:], et[:, 2 * T:3 * T])
                    with nc.allow_low_precision(reason="softmax probs in bf16"):
                        nc.vector.reciprocal(rz[:], z1[:])
                    pw = p3b.tile([H, 3, T], bf16, tag="pw", name="pw")
                    for w in range(3):
                        nc.vector.tensor_mul(pw[:, w, :],
                                             et[:, w * T:(w + 1) * T], rz[:])

                    for ch in range(8):
                        avs = []
                        for w in range(3):
                            bc = mm_ps.tile([P, T], f32, tag="mm", name=f"bc{ch}_{w}")
                            nc.tensor.matmul(
                                bc[:], emask[:, ch * P:(ch + 1) * P],
                                pw[:, w, :],
                                start=True, stop=True,
                            )
                            av = p3b.tile([P, T], bf16, tag="av", bufs=4,
                                          name=f"av{ch}_{w}")
                            nc.vector.tensor_mul(
                                av[:], bc[:],
                                kvT[:, (8 + ch) * TH + 2 - w:(8 + ch) * TH + TH - w],
                            )
                            avs.append(av)
                        av01 = p3b.tile([P, T], bf16, tag="av01", bufs=2,
                                        name=f"av01_{ch}")
                        nc.gpsimd.tensor_add(av01[:], avs[0][:], avs[1][:])
                        nc.vector.tensor_add(attnT[:, ch, :], av01[:], avs[2][:])

                # ---- proj + residual 1 + LN2 ----
                with tc.tile_pool(name="p5", bufs=1) as p5:
                    projw = p5.tile([P, KP, D], fp8, tag="projw", name="projw")
                    nc.sync.dma_start(projw[:], projw_d[:])
                    yT = p5.tile([P, 8 * T], bf16, tag="yT", name="yT")
                    for j in range(8):
                        ps = mm_ps.tile([P, T], f32, tag="mm", name=f"pj{j}")
                        dr_matmul(ps[:], projw, j * P, P, attnT, KP, 8)
                        nc.scalar.activation(yT[:, j * T:(j + 1) * T], ps[:],
                                             ACT.Identity,
                                             bias=projb[:, j:j + 1],
                                             scale=projs[:, j:j + 1])
                    for ti in range(4):
                        for g in range(2):
                            tpw = tp_ps.tile([P, 4, P], bf16, tag="tp",
                                             name=f"tpy{ti}_{g}")
                            for ch in range(4):
                                nc.tensor.transpose(
                                    tpw[:, ch, :],
                                    yT[:, (4 * g + ch) * T + ti * P:
                                       (4 * g + ch) * T + (ti + 1) * P],
                                    idb[:])
                            c0 = ti * D + g * 4 * P
                            nc.vector.tensor_add(
                                x2t[:, c0:c0 + 4 * P],
                                xt[:, c0:c0 + 4 * P], tpw[:])
                        layernorm_tok(x2t[:, ti * D:(ti + 1) * D], P, x2lnT,
                                      ti * P)

        # ---- MLP fc1 + gelu ----
        with tc.tile_pool(name="w1", bufs=1) as w1_pool:
            fc1w = w1_pool.tile([P, K1, HID], fp8, tag="fc1w", name="fc1w")
            nc.sync.dma_start(fc1w[:], fc1w_d[:])
            for j in range(32):
                ps = mm_ps.tile([P, T], f32, tag="mm", name=f"f1{j}")
                dr_matmul(ps[:], fc1w, j * P, P, x2lnT, K1, 8)
                nc.scalar.activation(hT[:, j, :], ps[:], ACT.Gelu,
                                     bias=fc1b[:, j:j + 1],
                                     scale=fc1s[:, j:j + 1])

            # ---- fc2 + residual 2 + store (overlaps fc1 tail) ----
            with tc.tile_pool(name="w2", bufs=1) as w2_pool:
                fc2w = w2_pool.tile([P, K2, D], fp8, tag="fc2w", name="fc2w")
                nc.sync.dma_start(fc2w[:], fc2w_d[:])
                outt = w2_pool.tile([P, 4 * D], f32, tag="outt", name="outt")
                mT = w2_pool.tile([P, 8 * T], bf16, tag="mT", name="mT")
                npairs = K2 // 2
                for j in range(8):
                    ps = sc_ps.tile([P, T], f32, tag="sc", name=f"f2{j}")
                    for i in range(npairs):
                        xc = (2 * i) % 32
                        nc.tensor.matmul(
                            ps[:],
                            fc2w[:, 2 * i:2 * i + 2, j * P:(j + 1) * P],
                            hT[:, xc:xc + 2, :],
                            start=(i == 0), stop=(i == npairs - 1),
                            perf_mode=DR,
                        )
                    nc.vector.tensor_scalar(mT[:, j * T:(j + 1) * T], ps[:],
                                            fc2s[:, j:j + 1], fc2b[:, j:j + 1],
                                            ALU.mult, ALU.add)
                for ti in range(4):
                    for g in range(2):
                        tpw = tp_ps.tile([P, 4, P], bf16, tag="tp",
                                         name=f"tpm{ti}_{g}")
                        for ch in range(4):
                            nc.tensor.transpose(
                                tpw[:, ch, :],
                                mT[:, (4 * g + ch) * T + ti * P:
                                   (4 * g + ch) * T + (ti + 1) * P],
                                idb[:])
                        c0 = ti * D + g * 4 * P
                        nc.vector.tensor_add(
                            outt[:, c0:c0 + 4 * P],
                            x2t[:, c0:c0 + 4 * P], tpw[:])
                    nc.sync.dma_start(out_d[ti * P:(ti + 1) * P, :],
                                      outt[:, ti * D:(ti + 1) * D])

    if not nc.is_finalized():
        nc.finalize()
    return nc


def _scale_w(w):
    """Per-column pow2 scale to absmax ~2. Returns (w_scaled, descale[cols])."""
    amax = np.abs(w).max(axis=0, keepdims=True)
    s = 2.0 ** np.round(np.log2(2.0 / np.maximum(amax, 1e-30)))
    return w * s, (1.0 / s)[0]


def _prep_w(w, comp):
    """[Din, Dout] fp32 -> ([128, kchunks*Dout] fp8 chunk-major hi(+lo),
    descale vector [Dout])."""
    din, dout = w.shape
    nch = din // P
    ws, descale = _scale_w(np.ascontiguousarray(w.astype(np.float32)))
    hi = ws.astype(F8)
    blocks = [hi]
    if comp:
        lo = (ws - hi.astype(np.float32)).astype(F8)
        blocks.append(lo)
    cols = []
    for b in blocks:
        cols.append(b.reshape(nch, P, dout).transpose(1, 0, 2))
    out = np.concatenate(cols, axis=1)  # [128, kchunks, dout]
    return np.ascontiguousarray(out.reshape(P, -1)), descale.astype(np.float32)


def _host_inputs(x, qkv_w, qkv_b, proj_w, proj_b, g1, b1, g2, b2,
                 fc1_w, fc1_b, fc2_w, fc2_b):
    """Build the 8 per-core input maps (fold LN affine + attn scale)."""
    scale = HD ** -0.5
    qkvw_eff = (qkv_w * g1[:, None]).astype(np.float32).copy()
    qkvb_eff = (qkv_b + b1 @ qkv_w).astype(np.float32).copy()
    qkvw_eff[:, 0:D] *= scale
    qkvb_eff[0:D] *= scale
    fc1w_eff = (fc1_w * g2[:, None]).astype(np.float32)
    fc1b_eff = (fc1_b + b2 @ fc1_w).astype(np.float32)

    qkvw_p, qkvs_v = _prep_w(qkvw_eff, COMP["qkv"])
    projw_p, projs_v = _prep_w(proj_w.astype(np.float32), COMP["proj"])
    fc1w_p, fc1s_v = _prep_w(fc1w_eff, COMP["fc1"])
    fc2w_p, fc2s_v = _prep_w(fc2_w.astype(np.float32), COMP["fc2"])

    # halo descale/bias: [128, 32] per (block j of k/v, col), feature-major
    khs_v = np.zeros((P, 32), np.float32)
    khb_v = np.zeros((P, 32), np.float32)
    kv_s = qkvs_v[D:3 * D].reshape(16, P)   # [block j, feature]
    kv_b = qkvb_eff[D:3 * D].reshape(16, P)
    for j in range(16):
        for c in range(2):
            khs_v[:, 2 * j + c] = kv_s[j]
            khb_v[:, 2 * j + c] = kv_b[j]

    common = {
        "qkvw": qkvw_p,
        "projw": projw_p,
        "fc1w": fc1w_p,
        "fc2w": fc2w_p,
        "qkvb": qkvb_eff.reshape(24, P).T.copy(),
        "qkvs": qkvs_v.reshape(24, P).T.copy(),
        "projb": proj_b.astype(np.float32).reshape(8, P).T.copy(),
        "projs": projs_v.reshape(8, P).T.copy(),
        "fc1b": fc1b_eff.reshape(32, P).T.copy(),
        "fc1s": fc1s_v.reshape(32, P).T.copy(),
        "fc2b": fc2_b.astype(np.float32).reshape(8, P).T.copy(),
        "fc2s": fc2s_v.reshape(8, P).T.copy(),
        "khs": khs_v,
        "khb": khb_v,
        "idb": np.eye(P, dtype=np.float32).astype(BF),
    }
    hm = np.zeros((P, 8, H), np.float32)
    for c in range(P):
        for ch in range(8):
            hm[c, ch, 2 * ch + c // HD] = 1.0
    common["hmask"] = hm.reshape(P, 8 * H).astype(BF)
    em = np.zeros((H, 8, P), np.float32)
    for ch in range(8):
        for m in range(P):
            em[2 * ch + m // HD, ch, m] = 1.0
    common["emask"] = em.reshape(H, 8 * P).astype(BF)

    in_maps = []
    for core in range(NCORE):
        b, q = divmod(core, 4)
        xm = np.ascontiguousarray(x[b, q * T:(q + 1) * T, :], dtype=np.float32)
        if q == 0:
            xhv = np.zeros((2, D), np.float32)
            emk = np.zeros((H, 3), np.float32)
        else:
            xhv = np.ascontiguousarray(x[b, q * T - 2:q * T, :], dtype=np.float32)
            emk = np.ones((H, 3), np.float32)
        m = dict(common)
        m["xm"] = xm
        m["xh"] = xhv
        m["emk"] = emk.astype(BF)
        in_maps.append(m)
    return in_maps


def kernel(**inputs) -> np.ndarray:
    from concourse.bass_utils import run_bass_kernel_spmd

    if "nc" not in _CACHE:
        _CACHE["nc"] = _build_program()
    nc = _CACHE["nc"]
    in_maps = _host_inputs(**inputs)
    res = run_bass_kernel_spmd(nc, in_maps, list(range(NCORE)))
    outs = res.results
    full = np.zeros((2, 2048, D), np.float32)
    for core in range(NCORE):
        b, q = divmod(core, 4)
        full[b, q * T:(q + 1) * T, :] = outs[core]["out"]
    return full


# revision 12
# speedup vs baseline: 1.9258x; 1.2200x over previous
"""Trainium2 Bass kernel: LocalCausalTransformerBlock (window-3 causal attention).

Sharding: 8-way sequence-parallel. B=2 x N=2048 = 4096 tokens -> 8 chunks of
512 tokens (4 chunks per batch row). Each core gets its 512 tokens plus a
2-token halo (the preceding tokens of the same sequence) so the window-3
causal attention needs no cross-core communication. Weights are replicated.

The four big matmuls (qkv/proj/fc1/fc2) run in fp8e4m3 with DoubleRow perf
mode (contract 2x128 channels per instruction at 0.5 cycles/row). Weights are
pre-scaled per output column to a power of two near absmax~2 so e4m3's
subnormal range is never hit; the descale rides the eviction's free
multiplicative scalar slot. qkv and proj additionally carry the quantization
residual ("lo") as extra fp8 k-chunks in the same accumulation group.
Attention internals (q/k/v, softmax, AV) are bf16; LayerNorm stats, softmax
normalizer and both residual streams are fp32.

Layout: activations live "transposed" (channels on partitions, tokens on the
free axis) so matmuls contract over partitions and the +-1/+-2 token shifts
of the local attention are free-axis offsets. LayerNorm runs token-major; PE
transposes bridge the layouts, batched 8-to-a-psum-bank with one wide strided
eviction. Softmax needs no max-subtraction (window-3 scores are small): exp
runs directly on the score PSUM; a per-core multiplicative edge mask zeroes
out-of-window columns after exp. Act-table funcs are ordered
sqrt->exp->sqrt->gelu (4 loads; identity is in every set). Weight matrices
stream in as column-block DMAs so matmuls start before the full matrix
lands; small constants ride in two packed DMAs. PSUM evictions are spread
across DVE and Act (gpsimd cannot touch PSUM); Pool takes SBUF-only work.
"""

import sys

for _p in ("/opt/trn_rl_repo",):
    if _p not in sys.path:
        sys.path.insert(0, _p)

import numpy as np
import ml_dtypes

P = 128
D = 1024
H = 16
HD = 64
H3 = 3 * D
HID = 4096
T = 512            # real tokens per core
TH = T + 2         # k/v token axis with 2-token halo (halo stored first)
NCORE = 8
EPS = 1e-5
BF = ml_dtypes.bfloat16
F8 = ml_dtypes.float8_e4m3

# which weights carry the fp8 quantization residual (2x k-chunks)
COMP = {"qkv": True, "proj": True, "fc1": False, "fc2": False}

# packed f32 const columns
_C = {}
_off = 0
for _name, _w in [("qkvb", 24), ("qkvs", 24), ("projb", 8), ("projs", 8),
                  ("fc1b", 32), ("fc1s", 32), ("fc2b", 8), ("fc2s", 8),
                  ("khs", 32), ("khb", 32)]:
    _C[_name] = _off
    _off += _w
CPAK_W = _off
# packed bf16 const columns: idb, hmask, emk, emask
_B = {"idb": 0, "hmask": 128, "emk": 256, "emask": 259}
BPAK_W = 259 + 1024

_CACHE: dict = {}


def _build_program():
    import concourse.bass as bass
    import concourse.tile as tile
    from concourse import bacc, mybir
    from contextlib import ExitStack

    f32 = mybir.dt.float32
    bf16 = mybir.dt.bfloat16
    fp8 = mybir.dt.float8e4
    ALU = mybir.AluOpType
    ACT = mybir.ActivationFunctionType
    DR = mybir.MatmulPerfMode.DoubleRow

    KQ = 16 if COMP["qkv"] else 8
    KP = 16 if COMP["proj"] else 8
    K1 = 16 if COMP["fc1"] else 8
    K2 = 64 if COMP["fc2"] else 32

    nc = bacc.Bacc()

    xh_d = nc.declare_dram_parameter("xh", [2, D], f32, isOutput=False)
    xm_d = nc.declare_dram_parameter("xm", [T, D], f32, isOutput=False)
    qkvw_ds = [nc.declare_dram_parameter(f"qkvw{b}", [P, KQ * 768], fp8,
                                         isOutput=False) for b in range(4)]
    projw_d = nc.declare_dram_parameter("projw", [P, KP * D], fp8, isOutput=False)
    fc1w_ds = [nc.declare_dram_parameter(f"fc1w{b}", [P, K1 * 2048], fp8,
                                         isOutput=False) for b in range(2)]
    fc2w_ds = [nc.declare_dram_parameter(f"fc2w{b}", [P, K2 * 512], fp8,
                                         isOutput=False) for b in range(2)]
    cpak_d = nc.declare_dram_parameter("cpak", [P, CPAK_W], f32, isOutput=False)
    bpak_d = nc.declare_dram_parameter("bpak", [P, BPAK_W], bf16, isOutput=False)
    out_d = nc.declare_dram_parameter("out", [T, D], f32, isOutput=True)

    with tile.TileContext(nc) as tc, ExitStack() as ctx:
        # PSUM budget (8 banks): mm x4, sc x2 (scores <-> fc2), tp x2
        const = ctx.enter_context(tc.tile_pool(name="const", bufs=1))
        acts = ctx.enter_context(tc.tile_pool(name="acts", bufs=1))
        ln_pool = ctx.enter_context(tc.tile_pool(name="ln", bufs=3))
        tp_ps = ctx.enter_context(tc.tile_pool(name="tp_ps", bufs=2, space="PSUM"))
        mm_ps = ctx.enter_context(tc.tile_pool(name="mm_ps", bufs=4, space="PSUM"))
        sc_ps = ctx.enter_context(tc.tile_pool(name="sc_ps", bufs=2, space="PSUM"))

        cpak = const.tile([P, CPAK_W], f32, tag="cp", name="cpak")
        nc.sync.dma_start(cpak[:], cpak_d[:])
        bpak = const.tile([P, BPAK_W], bf16, tag="bp", name="bpak")
        nc.sync.dma_start(bpak[:], bpak_d[:])

        def cp(name, j, w=1):
            o = _C[name] + j
            return cpak[:, o:o + w]

        idb = bpak[:, _B["idb"]:_B["idb"] + 128]
        hmask = bpak[:, _B["hmask"]:_B["hmask"] + 128]
        emk = bpak[0:H, _B["emk"]:_B["emk"] + 3]
        emask = bpak[0:H, _B["emask"]:_B["emask"] + 1024]

        # activations alive into the MLP phases
        x2t = acts.tile([P, 4 * D], f32, tag="x2t", name="x2t")
        x2lnT = acts.tile([P, 8, T], fp8, tag="x2lnT", name="x2lnT")
        hT = acts.tile([P, 32, T], fp8, tag="hT", name="hT")

        def layernorm_tok(src_ap, s, dstT, dst_off):
            """Token-major LN over s tokens -> fp8 channel-major in
            dstT[:, ch, dst_off:dst_off+s]. Stats on DVE, rstd via Act sqrt +
            DVE reciprocal, apply on Pool, transpose batch on PE, one wide
            DVE eviction."""
            stat = ln_pool.tile([s, 12], f32, tag=f"lnstat{s}", name=f"st{s}")
            nc.vector.bn_stats(stat[:, 0:6], src_ap[:, 0:512])
            nc.vector.bn_stats(stat[:, 6:12], src_ap[:, 512:1024])
            mv = ln_pool.tile([s, 2], f32, tag=f"lnmv{s}", name=f"mv{s}")
            nc.vector.bn_aggr(mv[:], stat[:])
            vpe = ln_pool.tile([s, 1], f32, tag=f"lnvpe{s}", name=f"vpe{s}")
            nc.vector.tensor_scalar_add(vpe[:], mv[:, 1:2], EPS)
            std = ln_pool.tile([s, 1], f32, tag=f"lnstd{s}", name=f"sd{s}")
            nc.scalar.activation(std[:], vpe[:], ACT.Sqrt)
            rstd = ln_pool.tile([s, 1], f32, tag=f"lnrstd{s}", name=f"rs{s}")
            nc.vector.reciprocal(rstd[:], std[:])
            nmr = ln_pool.tile([s, 1], f32, tag=f"lnnmr{s}", name=f"nm{s}")
            nc.vector.scalar_tensor_tensor(
                nmr[:], mv[:, 0:1], -1.0, rstd[:], ALU.mult, ALU.mult
            )
            xln = ln_pool.tile([s, D], bf16, tag=f"lnout{s}", name=f"xo{s}")
            nc.gpsimd.tensor_scalar(xln[:], src_ap[:], rstd[:, 0:1], nmr[:, 0:1],
                                    ALU.mult, ALU.add)
            tpw = tp_ps.tile([P, 8, s], bf16, tag="tp", name=f"tpln{s}")
            for ch in range(8):
                nc.tensor.transpose(tpw[:, ch, :], xln[:, ch * P:(ch + 1) * P],
                                    idb[0:s, 0:s])
            nc.vector.tensor_copy(dstT[:, :, dst_off:dst_off + s], tpw[:])

        with tc.tile_pool(name="p1", bufs=1) as p1:
            xt = p1.tile([P, 4 * D], f32, tag="xt", name="xt")
            xh = p1.tile([2, D], f32, tag="xh", name="xh")
            xlnT = p1.tile([P, 8, T], fp8, tag="xlnT", name="xlnT")
            xlnTh = p1.tile([P, 8, 2], fp8, tag="xlnTh", name="xlnTh")
            qT = p1.tile([P, 8 * T], bf16, tag="qT", name="qT")
            kvT = p1.tile([P, 16, TH], bf16, tag="kvT", name="kvT")

            for ti in range(4):
                nc.sync.dma_start(xt[:, ti * D:(ti + 1) * D],
                                  xm_d[ti * P:(ti + 1) * P, :])
            nc.sync.dma_start(xh[:], xh_d[:])

            with tc.tile_pool(name="wq", bufs=1) as wq_pool:
                qkvw = []
                for b in range(4):
                    t = wq_pool.tile([P, KQ, 768], fp8, tag=f"qkvw{b}",
                                     name=f"qkvw{b}")
                    nc.sync.dma_start(t[:], qkvw_ds[b][:])
                    qkvw.append(t)

                # ---- LN1 (halo + 4 token tiles) ----
                layernorm_tok(xh[:], 2, xlnTh, 0)
                for ti in range(4):
                    layernorm_tok(xt[:, ti * D:(ti + 1) * D], P, xlnT, ti * P)

                # ---- QKV ----
                # halo k/v columns: one psum tile = 16 blocks x 2 cols
                ph = tp_ps.tile([P, 8, 4], f32, tag="tp", name="ph")
                for j in range(16):
                    col = D + j * P
                    wt = qkvw[col // 768]
                    wo = col % 768
                    for i in range(KQ // 2):
                        xc = (2 * i) % 8
                        nc.tensor.matmul(
                            ph[:, j // 2, (j % 2) * 2:(j % 2) * 2 + 2],
                            wt[:, 2 * i:2 * i + 2, wo:wo + P],
                            xlnTh[:, xc:xc + 2, :],
                            start=(i == 0), stop=(i == KQ // 2 - 1),
                            perf_mode=DR,
                        )
                pht = ln_pool.tile([P, 32], f32, tag="pht", name="pht")
                nc.vector.tensor_mul(pht[:], ph[:, :, :], cp("khs", 0, 32))
                for j in range(16):
                    nc.gpsimd.tensor_add(
                        kvT[:, j, 0:2], pht[:, 2 * j:2 * j + 2],
                        cp("khb", 2 * j, 2))

                for j in range(24):
                    wt = qkvw[j // 6]
                    wo = (j % 6) * P
                    ps = mm_ps.tile([P, T], f32, tag="mm", name=f"qkv{j}")
                    for i in range(KQ // 2):
                        xc = (2 * i) % 8
                        nc.tensor.matmul(
                            ps[:], wt[:, 2 * i:2 * i + 2, wo:wo + P],
                            xlnT[:, xc:xc + 2, :],
                            start=(i == 0), stop=(i == KQ // 2 - 1),
                            perf_mode=DR,
                        )
                    if j < 8:
                        dst = qT[:, j * T:(j + 1) * T]
                    else:
                        dst = kvT[:, j - 8, 2:TH]
                    nc.vector.tensor_scalar(dst, ps[:], cp("qkvs", j),
                                            cp("qkvb", j), ALU.mult, ALU.add)

            # ---- attention ----
            with tc.tile_pool(name="p3", bufs=1) as p3:
                attnT = p3.tile([P, 8, T], fp8, tag="attnT", name="attnT")
                with tc.tile_pool(name="p3b", bufs=1) as p3b:
                    et = p3b.tile([H, 3, T], bf16, tag="et", name="et")
                    es = []
                    for w in range(3):
                        # e = q*k_shift, two quad-wide muls per w
                        e = p3b.tile([P, 4, T], bf16, tag=f"e{w}", bufs=1,
                                     name=f"e{w}")
                        e2 = p3b.tile([P, 4, T], bf16, tag=f"e2{w}", bufs=1,
                                      name=f"e2{w}")
                        nc.vector.tensor_mul(
                            e[:], qT[:, 0:4 * T], kvT[:, 0:4, 2 - w:2 - w + T])
                        nc.vector.tensor_mul(
                            e2[:], qT[:, 4 * T:8 * T],
                            kvT[:, 4:8, 2 - w:2 - w + T])
                        es.append((e, e2))
                    for w in range(3):
                        sc = sc_ps.tile([H, T], f32, tag="sc", name=f"sc{w}")
                        e, e2 = es[w]
                        for ch in range(8):
                            src = e if ch < 4 else e2
                            nc.tensor.matmul(
                                sc[:], hmask[:, ch * H:(ch + 1) * H],
                                src[:, ch % 4, :],
                                start=(ch == 0), stop=(ch == 7),
                            )
                        nc.scalar.activation(et[:, w, :], sc[:], ACT.Exp)
                    # zero out-of-window exp values on sequence-first chunks
                    nc.gpsimd.tensor_mul(et[:, 1, 0:1], et[:, 1, 0:1],
                                         emk[:, 0:1])
                    nc.gpsimd.tensor_mul(et[:, 2, 0:2], et[:, 2, 0:2],
                                         emk[:, 1:3])
                    z0 = p3b.tile([H, T], bf16, tag="z0", name="z0")
                    z1 = p3b.tile([H, T], bf16, tag="z1", name="z1")
                    rz = p3b.tile([H, T], bf16, tag="rz", name="rz")
                    nc.gpsimd.tensor_add(z0[:], et[:, 0, :], et[:, 1, :])
                    nc.gpsimd.tensor_add(z1[:], z0[:], et[:, 2, :])
                    with nc.allow_low_precision(reason="softmax probs in bf16"):
                        nc.vector.reciprocal(rz[:], z1[:])
                    pw = p3b.tile([H, 3, T], bf16, tag="pw", name="pw")
                    for w in range(3):
                        nc.vector.tensor_mul(pw[:, w, :], et[:, w, :], rz[:])

                    # broadcast probs to channels; evict to SBUF via Act so
                    # the AV muls run bf16 2x on SBUF
                    bcs = p3b.tile([P, 8, 3, T], bf16, tag="bcs", name="bcs")
                    for ch in range(8):
                        for w in range(3):
                            bc = mm_ps.tile([P, T], f32, tag="mm",
                                            name=f"bc{ch}_{w}")
                            nc.tensor.matmul(
                                bc[:], emask[:, ch * P:(ch + 1) * P],
                                pw[:, w, :], start=True, stop=True,
                            )
                            nc.scalar.activation(bcs[:, ch, w, :], bc[:],
                                                 ACT.Identity)
                    for chp in range(4):  # chunk pairs
                        ch = 2 * chp
                        avs = []
                        for w in range(3):
                            av = p3b.tile([P, 2, T], bf16, tag="av", bufs=6,
                                          name=f"av{chp}_{w}")
                            nc.vector.tensor_mul(
                                av[:], bcs[:, ch:ch + 2, w, :],
                                kvT[:, 8 + ch:10 + ch, 2 - w:2 - w + T],
                            )
                            avs.append(av)
                        av01 = p3b.tile([P, 2, T], bf16, tag="av01", bufs=2,
                                        name=f"av01_{chp}")
                        nc.gpsimd.tensor_add(av01[:], avs[0][:], avs[1][:])
                        nc.vector.tensor_add(attnT[:, ch:ch + 2, :], av01[:],
                                             avs[2][:])

                # ---- proj + residual 1 + LN2 ----
                with tc.tile_pool(name="p5", bufs=1) as p5:
                    projw = p5.tile([P, KP, D], fp8, tag="projw", name="projw")
                    nc.sync.dma_start(projw[:], projw_d[:])
                    yT = p5.tile([P, 8 * T], bf16, tag="yT", name="yT")
                    for j in range(8):
                        ps = mm_ps.tile([P, T], f32, tag="mm", name=f"pj{j}")
                        for i in range(KP // 2):
                            xc = (2 * i) % 8
                            nc.tensor.matmul(
                                ps[:], projw[:, 2 * i:2 * i + 2,
                                             j * P:(j + 1) * P],
                                attnT[:, xc:xc + 2, :],
                                start=(i == 0), stop=(i == KP // 2 - 1),
                                perf_mode=DR,
                            )
                        nc.scalar.activation(yT[:, j * T:(j + 1) * T], ps[:],
                                             ACT.Identity,
                                             bias=cp("projb", j),
                                             scale=cp("projs", j))
                    for ti in range(4):
                        for g in range(2):
                            tpw = tp_ps.tile([P, 4, P], bf16, tag="tp",
                                             name=f"tpy{ti}_{g}")
                            for ch in range(4):
                                nc.tensor.transpose(
                                    tpw[:, ch, :],
                                    yT[:, (4 * g + ch) * T + ti * P:
                                       (4 * g + ch) * T + (ti + 1) * P],
                                    idb[:, :])
                            c0 = ti * D + g * 4 * P
                            nc.vector.tensor_add(
                                x2t[:, c0:c0 + 4 * P],
                                xt[:, c0:c0 + 4 * P], tpw[:])
                        layernorm_tok(x2t[:, ti * D:(ti + 1) * D], P, x2lnT,
                                      ti * P)

        # ---- MLP fc1 + gelu, fc2 + residual 2 + store ----
        with tc.tile_pool(name="w1", bufs=1) as w1_pool:
            fc1w = []
            for b in range(2):
                t = w1_pool.tile([P, K1, 2048], fp8, tag=f"fc1w{b}",
                                 name=f"fc1w{b}")
                nc.sync.dma_start(t[:], fc1w_ds[b][:])
                fc1w.append(t)
            with tc.tile_pool(name="w2", bufs=1) as w2_pool:
                fc2w = []
                for b in range(2):
                    t = w2_pool.tile([P, K2, 512], fp8, tag=f"fc2w{b}",
                                     name=f"fc2w{b}")
                    nc.sync.dma_start(t[:], fc2w_ds[b][:])
                    fc2w.append(t)
                outt = w2_pool.tile([P, 4 * D], f32, tag="outt", name="outt")
                mT = w2_pool.tile([P, 8 * T], bf16, tag="mT", name="mT")

                for j in range(32):
                    wt = fc1w[j // 16]
                    wo = (j % 16) * P
                    ps = mm_ps.tile([P, T], f32, tag="mm", name=f"f1{j}")
                    for i in range(K1 // 2):
                        xc = (2 * i) % 8
                        nc.tensor.matmul(
                            ps[:], wt[:, 2 * i:2 * i + 2, wo:wo + P],
                            x2lnT[:, xc:xc + 2, :],
                            start=(i == 0), stop=(i == K1 // 2 - 1),
                            perf_mode=DR,
                        )
                    nc.scalar.activation(hT[:, j, :], ps[:], ACT.Gelu,
                                         bias=cp("fc1b", j),
                                         scale=cp("fc1s", j))

                for j in range(8):
                    wt = fc2w[j // 4]
                    wo = (j % 4) * P
                    ps = sc_ps.tile([P, T], f32, tag="sc", name=f"f2{j}")
                    for i in range(K2 // 2):
                        xc = (2 * i) % 32
                        nc.tensor.matmul(
                            ps[:], wt[:, 2 * i:2 * i + 2, wo:wo + P],
                            hT[:, xc:xc + 2, :],
                            start=(i == 0), stop=(i == K2 // 2 - 1),
                            perf_mode=DR,
                        )
                    nc.vector.tensor_scalar(mT[:, j * T:(j + 1) * T], ps[:],
                                            cp("fc2s", j), cp("fc2b", j),
                                            ALU.mult, ALU.add)
                for ti in range(4):
                    for g in range(2):
                        tpw = tp_ps.tile([P, 4, P], bf16, tag="tp",
                                         name=f"tpm{ti}_{g}")
                        for ch in range(4):
                            nc.tensor.transpose(
                                tpw[:, ch, :],
                                mT[:, (4 * g + ch) * T + ti * P:
                                   (4 * g + ch) * T + (ti + 1) * P],
                                idb[:, :])
                        c0 = ti * D + g * 4 * P
                        nc.vector.tensor_add(
                            outt[:, c0:c0 + 4 * P],
                            x2t[:, c0:c0 + 4 * P], tpw[:])
                    nc.sync.dma_start(out_d[ti * P:(ti + 1) * P, :],
                                      outt[:, ti * D:(ti + 1) * D])

    if not nc.is_finalized():
        nc.finalize()
    return nc


def _scale_w(w):
    amax = np.abs(w).max(axis=0, keepdims=True)
    s = 2.0 ** np.round(np.log2(2.0 / np.maximum(amax, 1e-30)))
    return w * s, (1.0 / s)[0]


def _prep_w(w, comp):
    """[Din, Dout] fp32 -> ([128, kchunks, Dout] fp8 chunk-major hi(+lo),
    descale vector [Dout])."""
    din, dout = w.shape
    nch = din // P
    ws, descale = _scale_w(np.ascontiguousarray(w.astype(np.float32)))
    hi = ws.astype(F8)
    blocks = [hi]
    if comp:
        lo = (ws - hi.astype(np.float32)).astype(F8)
        blocks.append(lo)
    cols = []
    for b in blocks:
        cols.append(b.reshape(nch, P, dout).transpose(1, 0, 2))
    out = np.concatenate(cols, axis=1)  # [128, kchunks, dout]
    return np.ascontiguousarray(out), descale.astype(np.float32)


def _host_inputs(x, qkv_w, qkv_b, proj_w, proj_b, g1, b1, g2, b2,
                 fc1_w, fc1_b, fc2_w, fc2_b):
    scale = HD ** -0.5
    qkvw_eff = (qkv_w * g1[:, None]).astype(np.float32).copy()
    qkvb_eff = (qkv_b + b1 @ qkv_w).astype(np.float32).copy()
    qkvw_eff[:, 0:D] *= scale
    qkvb_eff[0:D] *= scale
    fc1w_eff = (fc1_w * g2[:, None]).astype(np.float32)
    fc1b_eff = (fc1_b + b2 @ fc1_w).astype(np.float32)

    qkvw_p, qkvs_v = _prep_w(qkvw_eff, COMP["qkv"])
    projw_p, projs_v = _prep_w(proj_w.astype(np.float32), COMP["proj"])
    fc1w_p, fc1s_v = _prep_w(fc1w_eff, COMP["fc1"])
    fc2w_p, fc2s_v = _prep_w(fc2_w.astype(np.float32), COMP["fc2"])

    cpak = np.zeros((P, CPAK_W), np.float32)

    def setc(name, vec, n):
        cpak[:, _C[name]:_C[name] + n] = vec.reshape(n, P).T

    setc("qkvb", qkvb_eff, 24)
    setc("qkvs", qkvs_v, 24)
    setc("projb", proj_b.astype(np.float32), 8)
    setc("projs", projs_v, 8)
    setc("fc1b", fc1b_eff, 32)
    setc("fc1s", fc1s_v, 32)
    setc("fc2b", fc2_b.astype(np.float32), 8)
    setc("fc2s", fc2s_v, 8)
    kv_s = qkvs_v[D:3 * D].reshape(16, P)
    kv_b = qkvb_eff[D:3 * D].reshape(16, P)
    for j in range(16):
        for c in range(2):
            cpak[:, _C["khs"] + 2 * j + c] = kv_s[j]
            cpak[:, _C["khb"] + 2 * j + c] = kv_b[j]

    bpak0 = np.zeros((P, BPAK_W), np.float32)
    bpak0[:, _B["idb"]:_B["idb"] + 128] = np.eye(P)
    hm = np.zeros((P, 8, H), np.float32)
    for c in range(P):
        for ch in range(8):
            hm[c, ch, 2 * ch + c // HD] = 1.0
    bpak0[:, _B["hmask"]:_B["hmask"] + 128] = hm.reshape(P, 8 * H)
    em = np.zeros((H, 8, P), np.float32)
    for ch in range(8):
        for m in range(P):
            em[2 * ch + m // HD, ch, m] = 1.0
    bpak0[0:H, _B["emask"]:_B["emask"] + 1024] = em.reshape(H, 8 * P)

    common = {
        "projw": np.ascontiguousarray(projw_p.reshape(P, -1)),
        "cpak": cpak,
    }
    for b in range(4):
        common[f"qkvw{b}"] = np.ascontiguousarray(
            qkvw_p[:, :, b * 768:(b + 1) * 768].reshape(P, -1))
    for b in range(2):
        common[f"fc1w{b}"] = np.ascontiguousarray(
            fc1w_p[:, :, b * 2048:(b + 1) * 2048].reshape(P, -1))
    for b in range(2):
        common[f"fc2w{b}"] = np.ascontiguousarray(
            fc2w_p[:, :, b * 512:(b + 1) * 512].reshape(P, -1))

    in_maps = []
    for core in range(NCORE):
        b, q = divmod(core, 4)
        xm = np.ascontiguousarray(x[b, q * T:(q + 1) * T, :], dtype=np.float32)
        bpak = bpak0.copy()
        if q == 0:
            xhv = np.zeros((2, D), np.float32)
            # emk stays zero
        else:
            xhv = np.ascontiguousarray(x[b, q * T - 2:q * T, :], dtype=np.float32)
            bpak[0:H, _B["emk"]:_B["emk"] + 3] = 1.0
        m = dict(common)
        m["xm"] = xm
        m["xh"] = xhv
        m["bpak"] = bpak.astype(BF)
        in_maps.append(m)
    return in_maps


def kernel(**inputs) -> np.ndarray:
    from concourse.bass_utils import run_bass_kernel_spmd

    if "nc" not in _CACHE:
        _CACHE["nc"] = _build_program()
    nc = _CACHE["nc"]
    in_maps = _host_inputs(**inputs)
    res = run_bass_kernel_spmd(nc, in_maps, list(range(NCORE)))
    outs = res.results
    full = np.zeros((2, 2048, D), np.float32)
    for core in range(NCORE):
        b, q = divmod(core, 4)
        full[b, q * T:(q + 1) * T, :] = outs[core]["out"]
    return full


# revision 15
# speedup vs baseline: 2.1923x; 1.1384x over previous
"""Trainium2 Bass kernel: LocalCausalTransformerBlock (window-3 causal attention).

Sharding: 8-way sequence-parallel. B=2 x N=2048 = 4096 tokens -> 8 chunks of
512 tokens (4 chunks per batch row). Each core gets its 512 tokens plus a
2-token halo (the preceding tokens of the same sequence) so the window-3
causal attention needs no cross-core communication. Weights are replicated.

The four big matmuls (qkv/proj/fc1/fc2) run in fp8e4m3 with DoubleRow perf
mode (contract 2x128 channels per instruction at 0.5 cycles/row). Weights are
pre-scaled per output column to a power of two near absmax~2 so e4m3's
subnormal range is never hit; the descale rides the eviction's free
multiplicative scalar slot. qkv and proj additionally carry the quantization
residual ("lo") as extra fp8 k-chunks in the same accumulation group.
Attention internals (q/k/v, softmax, AV) are bf16; LayerNorm stats, softmax
normalizer and both residual streams are fp32.

Layout: activations live "transposed" (channels on partitions, tokens on the
free axis) so matmuls contract over partitions and the +-1/+-2 token shifts
of the local attention are free-axis offsets. LayerNorm runs token-major; PE
transposes bridge the layouts, batched 8-to-a-psum-bank with one wide strided
eviction. Softmax needs no max-subtraction (window-3 scores are small): exp
runs directly on the score PSUM; a per-core multiplicative edge mask zeroes
out-of-window columns after exp. Act-table funcs are ordered
sqrt->exp->sqrt->gelu (4 loads; identity is in every set). Weight matrices
stream in as column-block DMAs so matmuls start before the full matrix
lands; small constants ride in two packed DMAs. PSUM evictions are spread
across DVE and Act (gpsimd cannot touch PSUM); Pool takes SBUF-only work.
"""

import sys

for _p in ("/opt/trn_rl_repo",):
    if _p not in sys.path:
        sys.path.insert(0, _p)

import numpy as np
import ml_dtypes

P = 128
D = 1024
H = 16
HD = 64
H3 = 3 * D
HID = 4096
T = 512            # real tokens per core
TH = T + 2         # k/v token axis with 2-token halo (halo stored first)
NCORE = 8
EPS = 1e-5
BF = ml_dtypes.bfloat16
F8 = ml_dtypes.float8_e4m3

# which weights carry the fp8 quantization residual (2x k-chunks)
COMP = {"qkv": True, "proj": True, "fc1": False, "fc2": False}

# packed f32 const columns
_C = {}
_off = 0
for _name, _w in [("qkvb", 24), ("qkvs", 24), ("projb", 8), ("projs", 8),
                  ("fc1b", 32), ("fc1s", 32), ("fc2b", 8), ("fc2s", 8),
                  ("khs", 32), ("khb", 32)]:
    _C[_name] = _off
    _off += _w
CPAK_W = _off
# packed bf16 const columns: idb, hmask, emk, emask
_B = {"idb": 0, "hmask": 128, "emk": 256, "emask": 259}
BPAK_W = 259 + 1024

_CACHE: dict = {}


def _build_program():
    import concourse.bass as bass
    import concourse.tile as tile
    from concourse import bacc, mybir
    from contextlib import ExitStack

    f32 = mybir.dt.float32
    bf16 = mybir.dt.bfloat16
    fp8 = mybir.dt.float8e4
    ALU = mybir.AluOpType
    ACT = mybir.ActivationFunctionType
    DR = mybir.MatmulPerfMode.DoubleRow

    KQ = 16 if COMP["qkv"] else 8
    KP = 16 if COMP["proj"] else 8
    K1 = 16 if COMP["fc1"] else 8
    K2 = 64 if COMP["fc2"] else 32

    nc = bacc.Bacc()

    xh_d = nc.declare_dram_parameter("xh", [2, D], f32, isOutput=False)
    xm_d = nc.declare_dram_parameter("xm", [T, D], f32, isOutput=False)
    qkvw_ds = [nc.declare_dram_parameter(f"qkvw{b}", [P, KQ * 768], fp8,
                                         isOutput=False) for b in range(4)]
    projw_d = nc.declare_dram_parameter("projw", [P, KP * D], fp8, isOutput=False)
    fc1w_ds = [nc.declare_dram_parameter(f"fc1w{b}", [P, K1 * 2048], fp8,
                                         isOutput=False) for b in range(2)]
    fc2w_ds = [nc.declare_dram_parameter(f"fc2w{b}", [P, K2 * 512], fp8,
                                         isOutput=False) for b in range(2)]
    cpak_d = nc.declare_dram_parameter("cpak", [P, CPAK_W], f32, isOutput=False)
    bpak_d = nc.declare_dram_parameter("bpak", [P, BPAK_W], bf16, isOutput=False)
    out_d = nc.declare_dram_parameter("out", [T, D], f32, isOutput=True)

    with tile.TileContext(nc) as tc, ExitStack() as ctx:
        # PSUM budget (8 banks): mm x4, sc x2 (scores <-> fc2), tp x2
        const = ctx.enter_context(tc.tile_pool(name="const", bufs=1))
        acts = ctx.enter_context(tc.tile_pool(name="acts", bufs=1))
        ln_pool = ctx.enter_context(tc.tile_pool(name="ln", bufs=3))
        tp_ps = ctx.enter_context(tc.tile_pool(name="tp_ps", bufs=2, space="PSUM"))
        mm_ps = ctx.enter_context(tc.tile_pool(name="mm_ps", bufs=4, space="PSUM"))
        sc_ps = ctx.enter_context(tc.tile_pool(name="sc_ps", bufs=2, space="PSUM"))

        cpak = const.tile([P, CPAK_W], f32, tag="cp", name="cpak")
        nc.sync.dma_start(cpak[:], cpak_d[:])
        bpak = const.tile([P, BPAK_W], bf16, tag="bp", name="bpak")
        nc.sync.dma_start(bpak[:], bpak_d[:])

        def cp(name, j, w=1):
            o = _C[name] + j
            return cpak[:, o:o + w]

        idb = bpak[:, _B["idb"]:_B["idb"] + 128]
        hmask = bpak[:, _B["hmask"]:_B["hmask"] + 128]
        emk = bpak[0:H, _B["emk"]:_B["emk"] + 3]
        emask = bpak[0:H, _B["emask"]:_B["emask"] + 1024]

        # activations alive into the MLP phases
        x2t = acts.tile([P, 4 * D], f32, tag="x2t", name="x2t")
        x2lnT = acts.tile([P, 8, T], fp8, tag="x2lnT", name="x2lnT")
        hT = acts.tile([P, 32, T], fp8, tag="hT", name="hT")
        # weights preloaded early so their DMAs overlap earlier phases
        projw = acts.tile([P, KP, D], fp8, tag="projw", name="projw")
        fc1w = [acts.tile([P, K1, 2048], fp8, tag=f"fc1w{b}", name=f"fc1w{b}")
                for b in range(2)]

        def layernorm_tok(src_ap, s, dstT, dst_off):
            """Token-major LN over s tokens -> fp8 channel-major in
            dstT[:, ch, dst_off:dst_off+s]. Stats on DVE, rstd via Act sqrt +
            DVE reciprocal, apply on Pool, transpose batch on PE, one wide
            DVE eviction."""
            stat = ln_pool.tile([s, 12], f32, tag=f"lnstat{s}", name=f"st{s}")
            nc.vector.bn_stats(stat[:, 0:6], src_ap[:, 0:512])
            nc.vector.bn_stats(stat[:, 6:12], src_ap[:, 512:1024])
            mv = ln_pool.tile([s, 2], f32, tag=f"lnmv{s}", name=f"mv{s}")
            nc.vector.bn_aggr(mv[:], stat[:])
            vpe = ln_pool.tile([s, 1], f32, tag=f"lnvpe{s}", name=f"vpe{s}")
            nc.vector.tensor_scalar_add(vpe[:], mv[:, 1:2], EPS)
            std = ln_pool.tile([s, 1], f32, tag=f"lnstd{s}", name=f"sd{s}")
            nc.scalar.activation(std[:], vpe[:], ACT.Sqrt)
            rstd = ln_pool.tile([s, 1], f32, tag=f"lnrstd{s}", name=f"rs{s}")
            nc.vector.reciprocal(rstd[:], std[:])
            nmr = ln_pool.tile([s, 1], f32, tag=f"lnnmr{s}", name=f"nm{s}")
            nc.vector.scalar_tensor_tensor(
                nmr[:], mv[:, 0:1], -1.0, rstd[:], ALU.mult, ALU.mult
            )
            xln = ln_pool.tile([s, D], bf16, tag=f"lnout{s}", name=f"xo{s}")
            nc.gpsimd.tensor_scalar(xln[:], src_ap[:], rstd[:, 0:1], nmr[:, 0:1],
                                    ALU.mult, ALU.add)
            tpw = tp_ps.tile([P, 8, s], bf16, tag="tp", name=f"tpln{s}")
            for ch in range(8):
                nc.tensor.transpose(tpw[:, ch, :], xln[:, ch * P:(ch + 1) * P],
                                    idb[0:s, 0:s])
            nc.scalar.activation(dstT[:, :, dst_off:dst_off + s], tpw[:],
                                 ACT.Identity)

        with tc.tile_pool(name="p1", bufs=1) as p1:
            xt = p1.tile([P, 4 * D], f32, tag="xt", name="xt")
            xh = p1.tile([2, D], f32, tag="xh", name="xh")
            xlnT = p1.tile([P, 8, T], fp8, tag="xlnT", name="xlnT")
            xlnTh = p1.tile([P, 8, 2], fp8, tag="xlnTh", name="xlnTh")
            qT = p1.tile([P, 8 * T], bf16, tag="qT", name="qT")
            kvT = p1.tile([P, 16, TH], bf16, tag="kvT", name="kvT")

            for ti in range(4):
                nc.sync.dma_start(xt[:, ti * D:(ti + 1) * D],
                                  xm_d[ti * P:(ti + 1) * P, :])
            nc.sync.dma_start(xh[:], xh_d[:])

            with tc.tile_pool(name="wq", bufs=1) as wq_pool:
                qkvw = []
                for b in range(4):
                    t = wq_pool.tile([P, KQ, 768], fp8, tag=f"qkvw{b}",
                                     name=f"qkvw{b}")
                    nc.sync.dma_start(t[:], qkvw_ds[b][:])
                    qkvw.append(t)
                for b in range(2):
                    nc.sync.dma_start(fc1w[b][:], fc1w_ds[b][:])
                nc.sync.dma_start(projw[:], projw_d[:])

                # ---- LN1 (halo + 4 token tiles) ----
                layernorm_tok(xh[:], 2, xlnTh, 0)
                for ti in range(4):
                    layernorm_tok(xt[:, ti * D:(ti + 1) * D], P, xlnT, ti * P)

                # ---- QKV ----
                # halo k/v columns: one psum tile = 16 blocks x 2 cols
                ph = tp_ps.tile([P, 8, 4], f32, tag="tp", name="ph")
                for j in range(16):
                    col = D + j * P
                    wt = qkvw[col // 768]
                    wo = col % 768
                    for i in range(KQ // 2):
                        xc = (2 * i) % 8
                        nc.tensor.matmul(
                            ph[:, j // 2, (j % 2) * 2:(j % 2) * 2 + 2],
                            wt[:, 2 * i:2 * i + 2, wo:wo + P],
                            xlnTh[:, xc:xc + 2, :],
                            start=(i == 0), stop=(i == KQ // 2 - 1),
                            perf_mode=DR,
                        )
                pht = ln_pool.tile([P, 32], f32, tag="pht", name="pht")
                nc.vector.tensor_mul(pht[:], ph[:, :, :], cp("khs", 0, 32))
                for j in range(16):
                    nc.gpsimd.tensor_add(
                        kvT[:, j, 0:2], pht[:, 2 * j:2 * j + 2],
                        cp("khb", 2 * j, 2))

                for j in range(24):
                    wt = qkvw[j // 6]
                    wo = (j % 6) * P
                    ps = mm_ps.tile([P, T], f32, tag="mm", name=f"qkv{j}")
                    for i in range(KQ // 2):
                        xc = (2 * i) % 8
                        nc.tensor.matmul(
                            ps[:], wt[:, 2 * i:2 * i + 2, wo:wo + P],
                            xlnT[:, xc:xc + 2, :],
                            start=(i == 0), stop=(i == KQ // 2 - 1),
                            perf_mode=DR,
                        )
                    if j < 8:
                        dst = qT[:, j * T:(j + 1) * T]
                    else:
                        dst = kvT[:, j - 8, 2:TH]
                    if j % 2 == 0:
                        nc.vector.tensor_scalar(dst, ps[:], cp("qkvs", j),
                                                cp("qkvb", j), ALU.mult, ALU.add)
                    else:
                        nc.scalar.activation(dst, ps[:], ACT.Identity,
                                             bias=cp("qkvb", j),
                                             scale=cp("qkvs", j))

            # ---- attention ----
            with tc.tile_pool(name="p3", bufs=1) as p3:
                attnT = p3.tile([P, 8, T], fp8, tag="attnT", name="attnT")
                with tc.tile_pool(name="p3b", bufs=1) as p3b:
                    et = p3b.tile([H, 3, T], bf16, tag="et", name="et")
                    for w in range(3):
                        # e = q*k_shift, two quad-wide muls per w
                        e = p3b.tile([P, 4, T], bf16, tag="e", bufs=2,
                                     name=f"e{w}")
                        e2 = p3b.tile([P, 4, T], bf16, tag="e", bufs=2,
                                      name=f"e2{w}")
                        nc.vector.tensor_mul(
                            e[:], qT[:, 0:4 * T], kvT[:, 0:4, 2 - w:2 - w + T])
                        nc.vector.tensor_mul(
                            e2[:], qT[:, 4 * T:8 * T],
                            kvT[:, 4:8, 2 - w:2 - w + T])
                        sc = sc_ps.tile([H, T], f32, tag="sc", name=f"sc{w}")
                        for ch in range(8):
                            esrc = e if ch < 4 else e2
                            nc.tensor.matmul(
                                sc[:], hmask[:, ch * H:(ch + 1) * H],
                                esrc[:, ch % 4, :],
                                start=(ch == 0), stop=(ch == 7),
                            )
                        nc.scalar.activation(et[:, w, :], sc[:], ACT.Exp)
                    # zero out-of-window exp values on sequence-first chunks
                    nc.gpsimd.tensor_mul(et[:, 1, 0:1], et[:, 1, 0:1],
                                         emk[:, 0:1])
                    nc.gpsimd.tensor_mul(et[:, 2, 0:2], et[:, 2, 0:2],
                                         emk[:, 1:3])
                    z0 = p3b.tile([H, T], bf16, tag="z0", name="z0")
                    z1 = p3b.tile([H, T], bf16, tag="z1", name="z1")
                    rz = p3b.tile([H, T], bf16, tag="rz", name="rz")
                    nc.gpsimd.tensor_add(z0[:], et[:, 0, :], et[:, 1, :])
                    nc.gpsimd.tensor_add(z1[:], z0[:], et[:, 2, :])
                    with nc.allow_low_precision(reason="softmax probs in bf16"):
                        nc.vector.reciprocal(rz[:], z1[:])
                    pw = p3b.tile([H, 3, T], bf16, tag="pw", name="pw")
                    for w in range(3):
                        nc.vector.tensor_mul(pw[:, w, :], et[:, w, :], rz[:])

                    # broadcast probs to channels; evict to SBUF via Act so
                    # the AV muls run bf16 2x on SBUF
                    bcs = p3b.tile([P, 8, 3, T], bf16, tag="bcs", name="bcs")
                    for ch in range(8):
                        for w in range(3):
                            bc = mm_ps.tile([P, T], f32, tag="mm",
                                            name=f"bc{ch}_{w}")
                            nc.tensor.matmul(
                                bc[:], emask[:, ch * P:(ch + 1) * P],
                                pw[:, w, :], start=True, stop=True,
                            )
                            if ch % 4 == 3:
                                nc.vector.tensor_copy(bcs[:, ch, w, :], bc[:])
                            else:
                                nc.scalar.activation(bcs[:, ch, w, :], bc[:],
                                                     ACT.Identity)
                    for chp in range(4):  # chunk pairs
                        ch = 2 * chp
                        avs = []
                        for w in range(3):
                            av = p3b.tile([P, 2, T], bf16, tag="av", bufs=4,
                                          name=f"av{chp}_{w}")
                            nc.vector.tensor_mul(
                                av[:], bcs[:, ch:ch + 2, w, :],
                                kvT[:, 8 + ch:10 + ch, 2 - w:2 - w + T],
                            )
                            avs.append(av)
                        av01 = p3b.tile([P, 2, T], bf16, tag="av01", bufs=2,
                                        name=f"av01_{chp}")
                        nc.gpsimd.tensor_add(av01[:], avs[0][:], avs[1][:])
                        nc.vector.tensor_add(attnT[:, ch:ch + 2, :], av01[:],
                                             avs[2][:])

                # ---- proj + residual 1 + LN2 ----
                with tc.tile_pool(name="p5", bufs=1) as p5:
                    yT = p5.tile([P, 8 * T], bf16, tag="yT", name="yT")
                    for j in range(8):
                        ps = mm_ps.tile([P, T], f32, tag="mm", name=f"pj{j}")
                        for i in range(KP // 2):
                            xc = (2 * i) % 8
                            nc.tensor.matmul(
                                ps[:], projw[:, 2 * i:2 * i + 2,
                                             j * P:(j + 1) * P],
                                attnT[:, xc:xc + 2, :],
                                start=(i == 0), stop=(i == KP // 2 - 1),
                                perf_mode=DR,
                            )
                        nc.scalar.activation(yT[:, j * T:(j + 1) * T], ps[:],
                                             ACT.Identity,
                                             bias=cp("projb", j),
                                             scale=cp("projs", j))
                    for ti in range(4):
                        for g in range(2):
                            tpw = tp_ps.tile([P, 4, P], bf16, tag="tp",
                                             name=f"tpy{ti}_{g}")
                            for ch in range(4):
                                nc.tensor.transpose(
                                    tpw[:, ch, :],
                                    yT[:, (4 * g + ch) * T + ti * P:
                                       (4 * g + ch) * T + (ti + 1) * P],
                                    idb[:, :])
                            c0 = ti * D + g * 4 * P
                            nc.vector.tensor_add(
                                x2t[:, c0:c0 + 4 * P],
                                xt[:, c0:c0 + 4 * P], tpw[:])
                        layernorm_tok(x2t[:, ti * D:(ti + 1) * D], P, x2lnT,
                                      ti * P)

        # ---- MLP fc1 + gelu, fc2 + residual 2 + store ----
        with tc.tile_pool(name="w1", bufs=1) as w1_pool:
            with tc.tile_pool(name="w2", bufs=1) as w2_pool:
                fc2w = []
                for b in range(2):
                    t = w2_pool.tile([P, K2, 512], fp8, tag=f"fc2w{b}",
                                     name=f"fc2w{b}")
                    nc.sync.dma_start(t[:], fc2w_ds[b][:])
                    fc2w.append(t)
                outt = w2_pool.tile([P, 4 * D], f32, tag="outt", name="outt")
                mT = w2_pool.tile([P, 8 * T], bf16, tag="mT", name="mT")

                for j in range(32):
                    wt = fc1w[j // 16]
                    wo = (j % 16) * P
                    ps = mm_ps.tile([P, T], f32, tag="mm", name=f"f1{j}")
                    for i in range(K1 // 2):
                        xc = (2 * i) % 8
                        nc.tensor.matmul(
                            ps[:], wt[:, 2 * i:2 * i + 2, wo:wo + P],
                            x2lnT[:, xc:xc + 2, :],
                            start=(i == 0), stop=(i == K1 // 2 - 1),
                            perf_mode=DR,
                        )
                    nc.scalar.activation(hT[:, j, :], ps[:], ACT.Gelu,
                                         bias=cp("fc1b", j),
                                         scale=cp("fc1s", j))

                for j in range(8):
                    wt = fc2w[j // 4]
                    wo = (j % 4) * P
                    ps = sc_ps.tile([P, T], f32, tag="sc", name=f"f2{j}")
                    for i in range(K2 // 2):
                        xc = (2 * i) % 32
                        nc.tensor.matmul(
                            ps[:], wt[:, 2 * i:2 * i + 2, wo:wo + P],
                            hT[:, xc:xc + 2, :],
                            start=(i == 0), stop=(i == K2 // 2 - 1),
                            perf_mode=DR,
                        )
                    nc.scalar.activation(mT[:, j * T:(j + 1) * T], ps[:],
                                         ACT.Identity, bias=cp("fc2b", j),
                                         scale=cp("fc2s", j))
                for ti in range(4):
                    for g in range(2):
                        tpw = tp_ps.tile([P, 4, P], bf16, tag="tp",
                                         name=f"tpm{ti}_{g}")
                        for ch in range(4):
                            nc.tensor.transpose(
                                tpw[:, ch, :],
                                mT[:, (4 * g + ch) * T + ti * P:
                                   (4 * g + ch) * T + (ti + 1) * P],
                                idb[:, :])
                        c0 = ti * D + g * 4 * P
                        nc.vector.tensor_add(
                            outt[:, c0:c0 + 4 * P],
                            x2t[:, c0:c0 + 4 * P], tpw[:])
                    nc.sync.dma_start(out_d[ti * P:(ti + 1) * P, :],
                                      outt[:, ti * D:(ti + 1) * D])

    if not nc.is_finalized():
        nc.finalize()
    return nc


def _scale_w(w):
    amax = np.abs(w).max(axis=0, keepdims=True)
    s = 2.0 ** np.round(np.log2(2.0 / np.maximum(amax, 1e-30)))
    return w * s, (1.0 / s)[0]


def _prep_w(w, comp):
    """[Din, Dout] fp32 -> ([128, kchunks, Dout] fp8 chunk-major hi(+lo),
    descale vector [Dout])."""
    din, dout = w.shape
    nch = din // P
    ws, descale = _scale_w(np.ascontiguousarray(w.astype(np.float32)))
    hi = ws.astype(F8)
    blocks = [hi]
    if comp:
        lo = (ws - hi.astype(np.float32)).astype(F8)
        blocks.append(lo)
    cols = []
    for b in blocks:
        cols.append(b.reshape(nch, P, dout).transpose(1, 0, 2))
    out = np.concatenate(cols, axis=1)  # [128, kchunks, dout]
    return np.ascontiguousarray(out), descale.astype(np.float32)


def _host_inputs(x, qkv_w, qkv_b, proj_w, proj_b, g1, b1, g2, b2,
                 fc1_w, fc1_b, fc2_w, fc2_b):
    scale = HD ** -0.5
    qkvw_eff = (qkv_w * g1[:, None]).astype(np.float32).copy()
    qkvb_eff = (qkv_b + b1 @ qkv_w).astype(np.float32).copy()
    qkvw_eff[:, 0:D] *= scale
    qkvb_eff[0:D] *= scale
    fc1w_eff = (fc1_w * g2[:, None]).astype(np.float32)
    fc1b_eff = (fc1_b + b2 @ fc1_w).astype(np.float32)

    qkvw_p, qkvs_v = _prep_w(qkvw_eff, COMP["qkv"])
    projw_p, projs_v = _prep_w(proj_w.astype(np.float32), COMP["proj"])
    fc1w_p, fc1s_v = _prep_w(fc1w_eff, COMP["fc1"])
    fc2w_p, fc2s_v = _prep_w(fc2_w.astype(np.float32), COMP["fc2"])

    cpak = np.zeros((P, CPAK_W), np.float32)

    def setc(name, vec, n):
        cpak[:, _C[name]:_C[name] + n] = vec.reshape(n, P).T

    setc("qkvb", qkvb_eff, 24)
    setc("qkvs", qkvs_v, 24)
    setc("projb", proj_b.astype(np.float32), 8)
    setc("projs", projs_v, 8)
    setc("fc1b", fc1b_eff, 32)
    setc("fc1s", fc1s_v, 32)
    setc("fc2b", fc2_b.astype(np.float32), 8)
    setc("fc2s", fc2s_v, 8)
    kv_s = qkvs_v[D:3 * D].reshape(16, P)
    kv_b = qkvb_eff[D:3 * D].reshape(16, P)
    for j in range(16):
        for c in range(2):
            cpak[:, _C["khs"] + 2 * j + c] = kv_s[j]
            cpak[:, _C["khb"] + 2 * j + c] = kv_b[j]

    bpak0 = np.zeros((P, BPAK_W), np.float32)
    bpak0[:, _B["idb"]:_B["idb"] + 128] = np.eye(P)
    hm = np.zeros((P, 8, H), np.float32)
    for c in range(P):
        for ch in range(8):
            hm[c, ch, 2 * ch + c // HD] = 1.0
    bpak0[:, _B["hmask"]:_B["hmask"] + 128] = hm.reshape(P, 8 * H)
    em = np.zeros((H, 8, P), np.float32)
    for ch in range(8):
        for m in range(P):
            em[2 * ch + m // HD, ch, m] = 1.0
    bpak0[0:H, _B["emask"]:_B["emask"] + 1024] = em.reshape(H, 8 * P)

    common = {
        "projw": np.ascontiguousarray(projw_p.reshape(P, -1)),
        "cpak": cpak,
    }
    for b in range(4):
        common[f"qkvw{b}"] = np.ascontiguousarray(
            qkvw_p[:, :, b * 768:(b + 1) * 768].reshape(P, -1))
    for b in range(2):
        common[f"fc1w{b}"] = np.ascontiguousarray(
            fc1w_p[:, :, b * 2048:(b + 1) * 2048].reshape(P, -1))
    for b in range(2):
        common[f"fc2w{b}"] = np.ascontiguousarray(
            fc2w_p[:, :, b * 512:(b + 1) * 512].reshape(P, -1))

    in_maps = []
    for core in range(NCORE):
        b, q = divmod(core, 4)
        xm = np.ascontiguousarray(x[b, q * T:(q + 1) * T, :], dtype=np.float32)
        bpak = bpak0.copy()
        if q == 0:
            xhv = np.zeros((2, D), np.float32)
            # emk stays zero
        else:
            xhv = np.ascontiguousarray(x[b, q * T - 2:q * T, :], dtype=np.float32)
            bpak[0:H, _B["emk"]:_B["emk"] + 3] = 1.0
        m = dict(common)
        m["xm"] = xm
        m["xh"] = xhv
        m["bpak"] = bpak.astype(BF)
        in_maps.append(m)
    return in_maps


def kernel(**inputs) -> np.ndarray:
    from concourse.bass_utils import run_bass_kernel_spmd

    if "nc" not in _CACHE:
        _CACHE["nc"] = _build_program()
    nc = _CACHE["nc"]
    in_maps = _host_inputs(**inputs)
    res = run_bass_kernel_spmd(nc, in_maps, list(range(NCORE)))
    outs = res.results
    full = np.zeros((2, 2048, D), np.float32)
    for core in range(NCORE):
        b, q = divmod(core, 4)
        full[b, q * T:(q + 1) * T, :] = outs[core]["out"]
    return full
